# revision 1
# baseline (speedup 1.0000x reference)
"""Trainium2 Bass kernel for nn_AttentionSHA (dense transformer attention block).

Full inputs -> full output. Internally: tensor-parallel over heads across 8
NeuronCores (core g owns kv-head g and query heads 4g..4g+3; wo row-sharded),
host-side reduce of the 8 partial output projections.

Math notes (validated against the reference in fp64/fp32 numpy):
  - The reference adds a 0/1 causal mask *before* softmax (no -inf masking) and
    runs softmax over the full MAXSEQ=2048 cache axis where positions >= S hold
    zero k/v. Softmax without max-subtraction is exact here (scores are in
    [-17, 18]), so:  out = sum_t exp(sc_t)*m_t*v_t / (sum_t exp(sc_t)*m_t + 1024)
    with m_t = e if visible else 1, and +1024 = (MAXSEQ - S) zero-score tail.
    The e-factor for fully-visible regions folds into the Exp bias
    (exp(x + 1) = e*exp(x)); only the 128x128 diagonal blocks need a mask mult.
  - RoPE is applied via host-permuted weight rows (even channels then odd), a
    partition-half swap, and two multiply-adds against [cos;cos] / [-sin;sin].
"""
import numpy as np
from contextlib import ExitStack

S = 1024
D = 4096
NH = 32
NKV = 8
HD = 128
NREP = NH // NKV          # 4
MAXSEQ = 2048
NCORES = 8
DT = D // 128             # 32 d-tiles
TT = S // 128             # 8 t-tiles

_CACHE = {}


def _build_nc(phases=4, repeat=1):
    import concourse.bacc as bacc
    import concourse.mybir as mybir
    import concourse.tile as tile

    f32 = mybir.dt.float32
    f32r = mybir.dt.float32r
    Exp = mybir.ActivationFunctionType.Exp
    mult = mybir.AluOpType.mult
    add = mybir.AluOpType.add

    nc = bacc.Bacc("TRN2", target_bir_lowering=False, debug=False,
                   num_devices=NCORES)

    xT = nc.dram_tensor("xT", [D, S], f32r, kind="ExternalInput")
    wq_t = nc.dram_tensor("wq_t", [NREP, 128, DT * HD], f32r, kind="ExternalInput")
    wk_t = nc.dram_tensor("wk_t", [128, DT * HD], f32r, kind="ExternalInput")
    wv_t = nc.dram_tensor("wv_t", [128, DT * HD], f32r, kind="ExternalInput")
    wo_t = nc.dram_tensor("wo_t", [NREP * HD, D], f32r, kind="ExternalInput")
    cc_d = nc.dram_tensor("cc", [HD, S], f32, kind="ExternalInput")
    ns_d = nc.dram_tensor("ns", [HD, S], f32, kind="ExternalInput")
    emaskd_d = nc.dram_tensor("emaskd", [128, TT * 128], f32, kind="ExternalInput")
    ones_d = nc.dram_tensor("ones", [128, 128], f32r, kind="ExternalInput")
    ident_d = nc.dram_tensor("ident", [128, 128], f32, kind="ExternalInput")
    outT = nc.dram_tensor("outT", [D, S], f32, kind="ExternalOutput")

    with tile.TileContext(nc) as tc, ExitStack() as ctx:
        const = ctx.enter_context(tc.tile_pool(name="const", bufs=1))
        wts = ctx.enter_context(tc.tile_pool(name="wts", bufs=6))
        xpool = ctx.enter_context(tc.tile_pool(name="xpool", bufs=6))
        rpool = ctx.enter_context(tc.tile_pool(name="rpool", bufs=3))
        qkv = ctx.enter_context(tc.tile_pool(name="qkv", bufs=1))
        hs = ctx.enter_context(tc.tile_pool(name="hs", bufs=4))
        epool = ctx.enter_context(tc.tile_pool(name="epool", bufs=5))
        zpool = ctx.enter_context(tc.tile_pool(name="zpool", bufs=1))
        opool = ctx.enter_context(tc.tile_pool(name="opool", bufs=3))
        ps = ctx.enter_context(tc.tile_pool(name="ps", bufs=8, space="PSUM"))

        def _body():
            # ---- constants (loaded lazily at first use site) ----
            cc_sb = const.tile([128, S], f32)
            ns_sb = const.tile([128, S], f32)
            ones_sb = const.tile([128, 128], f32r)
            ident_sb = const.tile([128, 128], f32)
            emaskd_sb = const.tile([128, TT * 128], f32)

            # ---- weights; wo reuses these slots later ----
            # chunk DMAs are emitted inside the d-loop so x tiles interleave
            wq_sb = [wts.tile([128, D], f32r, name=f"wq_sb{h}", tag="w16")
                     for h in range(NREP)]
            wk_sb = wts.tile([128, D], f32r, tag="w16")
            wv_sb = wts.tile([128, D], f32r, tag="w16")

            WCHUNKS = [(d, 4) for d in range(0, DT, 4)]
            _wb = {d0: (d0, ln) for d0, ln in WCHUNKS}

            def load_w_chunk_span(d0, ln):
                c0, c1 = 128 * d0, 128 * (d0 + ln)
                for h in range(NREP):
                    nc.sync.dma_start(wq_sb[h][:, c0:c1], wq_t[h][:, c0:c1])
                nc.sync.dma_start(wk_sb[:, c0:c1], wk_t[:, c0:c1])
                nc.sync.dma_start(wv_sb[:, c0:c1], wv_t[:, c0:c1])

            if phases < 1:
                nul = const.tile([128, S], f32, name="nul")
                nc.sync.dma_start(nul[:], xT[0:128, :].bitcast(f32))
                nc.sync.dma_start(outT[0:128, :], nul[:])
                return
            # ---- phase 1: QKV projections + RoPE ----
            q_rot = [hs.tile([128, S], f32r, name=f"q_rot{h}", tag="hs")
                     for h in range(NREP)]                      # per head [e, s]
            k_rot = qkv.tile([128, S], f32r)                    # [e, t]
            v_et = qkv.tile([128, S], f32)                      # [e, t] pre-transpose
            v_te = qkv.tile([128, TT * 128], f32r)              # tile t: [t-part, e]

            for sh in range(2):
                s0 = 512 * sh
                q_ps = [ps.tile([128, 512], f32, tag="ps", name=f"q_ps{sh}_{h}")
                        for h in range(NREP)]
                k_ps = ps.tile([128, 512], f32, tag="ps", name=f"k_ps{sh}")
                v_ps = ps.tile([128, 512], f32, tag="ps", name=f"v_ps{sh}")
                for d in range(DT):
                    x_r = xpool.tile([128, 512], f32r, name="x_r")
                    nc.sync.dma_start(x_r[:], xT[128 * d:128 * (d + 1), s0:s0 + 512])
                    if sh == 0 and d in _wb:
                        load_w_chunk_span(*_wb[d])
                    for h in range(NREP):
                        nc.tensor.matmul(q_ps[h][:], wq_sb[h][:, 128 * d:128 * (d + 1)],
                                         x_r[:], start=(d == 0), stop=(d == DT - 1))
                    nc.tensor.matmul(k_ps[:], wk_sb[:, 128 * d:128 * (d + 1)],
                                     x_r[:], start=(d == 0), stop=(d == DT - 1))
                    nc.tensor.matmul(v_ps[:], wv_sb[:, 128 * d:128 * (d + 1)],
                                     x_r[:], start=(d == 0), stop=(d == DT - 1))

                if sh == 0:
                    nc.sync.dma_start(cc_sb[:], cc_d[:])
                    nc.sync.dma_start(ns_sb[:], ns_d[:])

                # RoPE: dest = psum*[cos;cos] + swap(psum)*[-sin;sin].
                # fast=True splits the swap copies across ACT+DVE — used for
                # q0 and k, whose rope latency gates phase 3's first scores
                def rope(psum, dest, fast=False):
                    sw = rpool.tile([128, 512], f32, name="sw")
                    if fast:
                        nc.vector.tensor_copy(sw[0:64, :], psum[64:128, :])
                    else:
                        nc.scalar.copy(sw[0:64, :], psum[64:128, :])
                    nc.scalar.copy(sw[64:128, :], psum[0:64, :])
                    t1 = rpool.tile([128, 512], f32, name="t1")
                    nc.vector.tensor_tensor(t1[:], psum[:], cc_sb[:, s0:s0 + 512], op=mult)
                    t2 = rpool.tile([128, 512], f32, name="t2")
                    nc.gpsimd.tensor_tensor(t2[:], sw[:], ns_sb[:, s0:s0 + 512], op=mult)
                    nc.vector.tensor_tensor(dest, t1[:], t2[:], op=add)

                nc.vector.tensor_copy(v_et[:, s0:s0 + 512], v_ps[:])
                rope(q_ps[0], q_rot[0][:, s0:s0 + 512], fast=(sh == 1))
                rope(k_ps, k_rot[:, s0:s0 + 512], fast=(sh == 1))
                for h in range(1, NREP):
                    rope(q_ps[h], q_rot[h][:, s0:s0 + 512], fast=(sh == 1))
                if phases >= 2 and (sh == 0 or phases == 2):
                    if sh == 0:
                        nc.sync.dma_start(ident_sb[:], ident_d[:])
                    for t in range(4 * sh, 4 * (sh + 1)):
                        tr = ps.tile([128, 128], f32, tag="ps", name="tr")
                        nc.tensor.transpose(tr[:], v_et[:, 128 * t:128 * (t + 1)],
                                            ident_sb[:])
                        nc.vector.tensor_copy(v_te[:, 128 * t:128 * (t + 1)], tr[:])

            if phases < 2:
                nc.sync.dma_start(outT[0:128, :], v_et[:])

            if phases == 2:
                nc.sync.dma_start(outT[0:128, :], v_te[:].bitcast(f32))
            # ---- phase 3: attention per head ----
            att = []                                  # per head [e, s], normalized
            inv_sqrt_hd = float(1.0 / np.sqrt(HD))
            if phases >= 3:
                nc.sync.dma_start(ones_sb[:], ones_d[:])
                nc.sync.dma_start(emaskd_sb[:], emaskd_d[:])
                # s-half-1 V transposes, deferred from phase 1: not consumed
                # until head 0's t=4 PV matmul, so they overlap the first
                # scores/exp instead of blocking phase 3 behind the rope queue
                for t in range(4, TT):
                    tr = ps.tile([128, 128], f32, tag="ps", name="tr")
                    nc.tensor.transpose(tr[:], v_et[:, 128 * t:128 * (t + 1)],
                                        ident_sb[:])
                    nc.scalar.copy(v_te[:, 128 * t:128 * (t + 1)], tr[:])
            for h in range(NREP if phases >= 3 else 0):
                z_ps = [ps.tile([128, 512], f32, tag="ps", name=f"z_ps{h}_{c}")
                        for c in range(2)]
                o_ps = [ps.tile([128, 512], f32, tag="ps", name=f"o_ps{h}_{c}")
                        for c in range(2)]
                def emit_sc_exp(t):
                    dlo, dhi = 128 * t, 128 * (t + 1)
                    expm = epool.tile([128, S], f32r, name="expm")
                    for c in range(2):
                        sc = ps.tile([128, 512], f32, tag="ps", name="sc")
                        nc.tensor.matmul(sc[:], k_rot[:, dlo:dhi],
                                         q_rot[h][:, 512 * c:512 * (c + 1)],
                                         start=True, stop=True)
                        lo, hi = 512 * c, 512 * (c + 1)
                        if dlo >= hi:
                            # fully invisible: plain exp
                            nc.scalar.activation(expm[:, lo:hi], sc[:], Exp,
                                                 scale=inv_sqrt_hd)
                        elif dhi <= lo:
                            # fully visible: exp(x + 1) = e * exp(x)
                            nc.scalar.activation(expm[:, lo:hi], sc[:], Exp,
                                                 scale=inv_sqrt_hd, bias=1.0)
                        else:
                            # diagonal block inside this chunk: one exp call,
                            # then the mask factors applied in-place (diag x
                            # emaskd on GpSimd; visible remainder x e on DVE)
                            nc.scalar.activation(expm[:, lo:hi], sc[:], Exp,
                                                 scale=inv_sqrt_hd)
                            nc.gpsimd.tensor_tensor(
                                expm[:, dlo:dhi], expm[:, dlo:dhi],
                                emaskd_sb[:, 128 * t:128 * (t + 1)], op=mult)
                            if dhi < hi:
                                nc.gpsimd.tensor_scalar_mul(
                                    expm[:, dhi:hi], expm[:, dhi:hi],
                                    float(np.e))
                    return expm

                pend = [emit_sc_exp(0), emit_sc_exp(1)]
                for t in range(TT):
                    if t + 2 < TT:
                        pend.append(emit_sc_exp(t + 2))
                    expm_t = pend.pop(0)
                    # z pair then o pair: the stationary operand (ones / v_te
                    # tile) is reused by consecutive matmuls, and the two psum
                    # groups still alternate within each pair
                    for c in range(2):
                        nc.tensor.matmul(z_ps[c][:], ones_sb[:],
                                         expm_t[:, 512 * c:512 * (c + 1)],
                                         start=(t == 0), stop=(t == TT - 1))
                    for c in range(2):
                        nc.tensor.matmul(o_ps[c][:], v_te[:, 128 * t:128 * (t + 1)],
                                         expm_t[:, 512 * c:512 * (c + 1)],
                                         start=(t == 0), stop=(t == TT - 1))
                z_sb = zpool.tile([128, S], f32, name="z_sb")
                rz = zpool.tile([128, S], f32, name="rz")
                a = hs.tile([128, S], f32r, name=f"att{h}", tag="hs")
                for c in range(2):
                    nc.vector.tensor_scalar_add(z_sb[:, 512 * c:512 * (c + 1)],
                                                z_ps[c][:], float(MAXSEQ - S))
                    nc.vector.reciprocal(rz[:, 512 * c:512 * (c + 1)],
                                         z_sb[:, 512 * c:512 * (c + 1)])
                    nc.vector.tensor_tensor(a[:, 512 * c:512 * (c + 1)],
                                            o_ps[c][:], rz[:, 512 * c:512 * (c + 1)],
                                            op=mult)
                att.append(a)

            if phases == 3:
                for h in range(NREP):
                    nc.sync.dma_start(outT[128 * h:128 * (h + 1), :], att[h][:].bitcast(f32))
            # ---- phase 4: output projection (partial over this core's 512 cols) ----
            wo_sb = []
            for h in range(NREP if phases >= 4 else 0):
                w = wts.tile([128, D], f32r, name=f"wo_sb{h}", tag="w16")
                nc.sync.dma_start(w[:], wo_t[128 * h:128 * (h + 1), :])
                wo_sb.append(w)

            for do in range(DT if phases >= 4 else 0):
                op_ps = [ps.tile([128, 512], f32, tag="ps", name=f"op{c}")
                         for c in range(2)]
                for h in range(NREP):
                    for c in range(2):
                        nc.tensor.matmul(op_ps[c][:],
                                         wo_sb[h][:, 128 * do:128 * (do + 1)],
                                         att[h][:, 512 * c:512 * (c + 1)],
                                         start=(h == 0), stop=(h == NREP - 1))
                out_sb = opool.tile([128, S], f32, name="out_sb")
                nc.vector.tensor_copy(out_sb[:, 0:512], op_ps[0][:])
                nc.scalar.copy(out_sb[:, 512:1024], op_ps[1][:])
                nc.sync.dma_start(outT[128 * do:128 * (do + 1), :], out_sb[:])


        for _rep in range(repeat):
            _body()

    nc.compile()
    return nc


def _to_f32r(x):
    """Host replica of the device fp32 -> fp32r conversion: round-to-nearest-
    even to an 11-bit mantissa (low 12 bits zeroed). Verified bit-exact against
    the DVE/DMA converters."""
    xi = np.ascontiguousarray(x, np.float32).view(np.uint32).astype(np.uint64)
    r = ((xi + 0x7FF + ((xi >> 12) & 1)) >> 12) << 12
    return (r & 0xFFFFFFFF).astype(np.uint32).view(np.float32)


def kernel(**inputs):
    from concourse.bass_utils import run_bass_kernel_spmd

    x = np.asarray(inputs["x"], np.float32)                 # [1, S, D]
    cos = np.asarray(inputs["freqs_cos"], np.float32)       # [S, 64]
    sin = np.asarray(inputs["freqs_sin"], np.float32)       # [S, 64]
    wq = np.asarray(inputs["wq"], np.float32)               # [NH, HD, D]
    wk = np.asarray(inputs["wk"], np.float32)               # [NKV, HD, D]
    wv = np.asarray(inputs["wv"], np.float32)               # [NKV, HD, D]
    wo = np.asarray(inputs["wo"], np.float32)               # [D, D]
    input_pos = np.asarray(inputs["input_pos"]).astype(np.int64)  # [S]

    if "nc" not in _CACHE:
        _CACHE["nc"] = _build_nc()
    nc = _CACHE["nc"]

    perm = np.concatenate([np.arange(0, HD, 2), np.arange(1, HD, 2)])
    xT = _to_f32r(x[0].T)                                   # [D, S] fp32r-encoded
    cc = np.ascontiguousarray(np.concatenate([cos.T, cos.T], 0))   # [128, S]
    ns = np.ascontiguousarray(np.concatenate([-sin.T, sin.T], 0))  # [128, S]
    # visibility adds +1 pre-exp where input_pos[t] <= input_pos[s]; for the
    # (spec-guaranteed) sorted arange fill only diagonal blocks are mixed.
    emaskd_t = np.empty((TT, 128, 128), np.float32)
    for t in range(TT):
        p = input_pos[128 * t:128 * (t + 1)]
        emaskd_t[t] = np.where(p[:, None] <= p[None, :], np.float32(np.e),
                               np.float32(1.0))
    # partition-major [128, TT*128] so the single DMA reads 4KB runs
    emaskd = np.ascontiguousarray(
        emaskd_t.transpose(1, 0, 2).reshape(128, TT * 128))
    ones128 = np.ones((128, 128), np.float32)
    ident = np.eye(128, dtype=np.float32)

    in_maps = []
    for g in range(NCORES):
        wq_g = wq[NREP * g:NREP * (g + 1)][:, perm, :]       # [4, 128, D]

        def pmajor(wT):
            # [D, 128e] -> [128p, DT*128e]: partition-major so each chunk DMA
            # reads 2KB-contiguous runs per partition
            return np.ascontiguousarray(
                wT.reshape(DT, 128, HD).transpose(1, 0, 2).reshape(128, DT * HD))

        in_maps.append({
            "xT": xT,
            "wq_t": _to_f32r(np.stack([pmajor(wq_g[j].T) for j in range(NREP)])),
            "wk_t": _to_f32r(pmajor(wk[g][perm].T)),     # [128, DT*128]
            "wv_t": _to_f32r(pmajor(wv[g].T)),           # [128, DT*128]
            "wo_t": _to_f32r(
                wo[:, NREP * HD * g:NREP * HD * (g + 1)].T),         # [512, D]
            "cc": cc, "ns": ns, "emaskd": emaskd,
            "ones": _to_f32r(ones128), "ident": ident,
        })

    res = run_bass_kernel_spmd(nc, in_maps, list(range(NCORES)))
    total = np.zeros((D, S), np.float64)
    for g in range(NCORES):
        total += res.results[g]["outT"]
    return np.ascontiguousarray(total.T.astype(np.float32)[None])   # [1, S, D]



# revision 38
# speedup vs baseline: 1.0830x; 1.0830x over previous
"""Trainium2 Bass kernel for nn_AttentionSHA (dense transformer attention block).

Full inputs -> full output. Tensor-parallel over heads across 8 NeuronCores
(core g owns kv-head g and query heads 4g..4g+3; wo row-sharded), host-side
reduce of the 8 partial output projections.

v2 design (fused pipeline):
  - Everything DMA'd travels as bf16 (x, wq/wk/wv, wo, trig, emask, out).
    TRN2 matmul cost tracks the moving operand; bf16 moves at 1 cyc/row at
    any width. Host-side simulation of the bf16 input rounding measured
    6.2e-3 max-rel error (gate 2e-2; device fp32r noise adds ~5e-4).
  - Phase A projects k, q0 and (lagged by 8 d-tiles, so the wv DMA can
    trail) v for both halves; RoPE runs on ACT/DVE/Pool behind the PE.
  - Pipeline slots: slot h runs head h's attention while also projecting
    head h+1's q on the PE. Per-head attention alone is ACT-bound (16 exps
    at ~0.6us) once z is off the PE, so the q-projection keeps PE busy.
  - Softmax denominator z via stationary-side trick: matmuls with expm
    slices *stationary* and a [128,1] ones moving vector accumulate
    z[s-block] columns at ~1 cycle each (cost follows output moving size).
    rz = 1/(z+1024) is transposed (PE) and re-broadcast across partitions
    with eight [1,128]-moving matmuls against a [1,128] ones stationary.
  - The reference adds a 0/1 causal mask *before* softmax (no -inf) and
    softmaxes over MAXSEQ=2048 whose tail positions hold zero k/v, hence
    z = sum_t exp(sc)*m_t + 1024 with m_t = e if visible else 1; the
    e-factor folds into the Exp bias except on 128x128 diagonal blocks.
"""
import numpy as np
from contextlib import ExitStack

S = 1024
D = 4096
NH = 32
NKV = 8
HD = 128
NREP = NH // NKV          # 4
MAXSEQ = 2048
NCORES = 8
DT = D // 128             # 32 d-tiles
TT = S // 128             # 8 t-tiles
VLAG = 8                  # phase-A v-stream lag in d-tiles

_CACHE = {}


def _build_nc(phases=4, repeat=1):
    import concourse.bacc as bacc
    import concourse.mybir as mybir
    import concourse.tile as tile

    f32 = mybir.dt.float32
    f32r = mybir.dt.float32r
    bf16 = mybir.dt.bfloat16
    Exp = mybir.ActivationFunctionType.Exp
    mult = mybir.AluOpType.mult
    add = mybir.AluOpType.add

    nc = bacc.Bacc("TRN2", target_bir_lowering=False, debug=False,
                   num_devices=NCORES)

    # x packed sh-major: col = sh*16384 + d*512 + s_local
    xT = nc.dram_tensor("xT", [128, 2 * DT * 512], bf16, kind="ExternalInput")
    wq_t = nc.dram_tensor("wq_t", [NREP, 128, DT * HD], bf16, kind="ExternalInput")
    wk_t = nc.dram_tensor("wk_t", [128, DT * HD], bf16, kind="ExternalInput")
    wv_t = nc.dram_tensor("wv_t", [128, DT * HD], bf16, kind="ExternalInput")
    # wo packed per head then d-major: col = h*D + do*128 + e ... see host
    wo_t = nc.dram_tensor("wo_t", [128, NREP * D], bf16, kind="ExternalInput")
    cc_d = nc.dram_tensor("cc", [HD, S], bf16, kind="ExternalInput")
    ns_d = nc.dram_tensor("ns", [HD, S], bf16, kind="ExternalInput")
    emaskd_d = nc.dram_tensor("emaskd", [128, TT * 128], bf16, kind="ExternalInput")
    ident_d = nc.dram_tensor("ident", [128, 128], f32, kind="ExternalInput")
    outT = nc.dram_tensor("outT", [D, S], bf16, kind="ExternalOutput")

    inv_sqrt_hd = float(1.0 / np.sqrt(HD))

    with tile.TileContext(nc) as tc, ExitStack() as ctx:
        const = ctx.enter_context(tc.tile_pool(name="const", bufs=1))
        big = ctx.enter_context(tc.tile_pool(name="big", bufs=1))
        wts = ctx.enter_context(tc.tile_pool(name="wts", bufs=1))
        hs = ctx.enter_context(tc.tile_pool(name="hs", bufs=1))
        rpool = ctx.enter_context(tc.tile_pool(name="rpool", bufs=2))
        epool = ctx.enter_context(tc.tile_pool(name="epool", bufs=1))
        zpool = ctx.enter_context(tc.tile_pool(name="zpool", bufs=1))
        opool = ctx.enter_context(tc.tile_pool(name="opool", bufs=2))
        ps = ctx.enter_context(tc.tile_pool(name="ps", bufs=1, space="PSUM"))

        def _body():
            # ---- persistent PSUM banks, hand-assigned ----
            # phase A: sh0 {k:P0, q0:P1, v:P2}, sh1 {k:P3, q0:P4, v:P5}
            # slots:   sc {P6,P7}, o_ps {P0,P1}, z/zT {P2},
            #          qproj {sh0:P5, sh1:P4}, rz broadcast {P3, P2},
            #          slot-0 v-transposes {P3,P4}[:,128:256]
            # phase 4: op pairs {P6,P7} / {P0,P1}
            P = [ps.tile([128, 512], f32, name=f"bankP{i}", tag=f"bankP{i}")
                 for i in range(8)]

            # ---- constants ----
            cc_sb = const.tile([128, S], bf16)
            ns_sb = const.tile([128, S], bf16)
            ident_sb = const.tile([128, 128], f32)
            emaskd_sb = const.tile([128, TT * 128], bf16)
            onec_sb = const.tile([128, 1], bf16)     # z moving vector
            oner_sb = const.tile([1, 128], bf16)     # rz-broadcast stationary
            nc.gpsimd.memset(onec_sb[:], 1.0)
            nc.gpsimd.memset(oner_sb[:], 1.0)

            # ---- SBUF tensors ----
            x_sb = big.tile([128, 2 * DT * 512], bf16)
            wq_sb = [wts.tile([128, D], bf16, name=f"wq_sb{h}", tag=f"wq{h}")
                     for h in range(NREP)]
            wk_sb = wts.tile([128, D], bf16, tag="wk")
            wv_sb = wts.tile([128, D], bf16, tag="wv")
            # wo streamed in halves: tag wo{h} rotates 2 bufs of [128, 16*128]
            wo_half = {}

            q_rot = [hs.tile([128, S], bf16, name=f"q_rot{h}", tag=f"qr{h}")
                     for h in range(NREP)]
            k_rot = hs.tile([128, S], bf16, tag="kr")
            v_et = hs.tile([128, S], f32, tag="vet")     # [e, t] pre-transpose
            v_te = hs.tile([128, TT * 128], bf16, tag="vte")  # tile t: [t, e]
            a = [hs.tile([128, S], bf16, name=f"a{h}", tag=f"a{h}")
                 for h in range(NREP)]
            expmb = [epool.tile([128, S], bf16, name=f"expmb{i}", tag=f"eb{i}")
                     for i in range(4)]

            def xs(sh, d):
                c = sh * 16384 + d * 512
                return x_sb[:, c:c + 512]

            def issue_dma(ev):
                kind = ev[0]
                if kind == "x":
                    sh, d0, d1 = ev[1], ev[2], ev[3]
                    c0, c1 = sh * 16384 + d0 * 512, sh * 16384 + d1 * 512
                    nc.sync.dma_start(x_sb[:, c0:c1], xT[:, c0:c1])
                elif kind in ("wk", "wv") or kind.startswith("wq"):
                    w_sb, w_d = {"wk": (wk_sb, wk_t), "wv": (wv_sb, wv_t)}.get(
                        kind, (None, None))
                    if w_sb is None:
                        h = int(kind[2])
                        w_sb, w_d = wq_sb[h], wq_t[h]
                    d0, d1 = ev[1], ev[2]
                    nc.sync.dma_start(w_sb[:, 128 * d0:128 * d1],
                                      w_d[:, 128 * d0:128 * d1])
                elif kind == "wo":
                    h, half = ev[1], ev[2]
                    t_ = wts.tile([128, 16 * 128], bf16, name=f"wo{h}_{half}",
                                  tag=f"wo{h}", bufs=2)
                    wo_half[(h, half)] = t_
                    c0 = D * h + 2048 * half
                    nc.sync.dma_start(t_[:], wo_t[:, c0:c0 + 2048])
                elif kind == "cc":
                    nc.sync.dma_start(cc_sb[:], cc_d[:])
                elif kind == "ns":
                    nc.sync.dma_start(ns_sb[:], ns_d[:])
                elif kind == "emaskd":
                    nc.sync.dma_start(emaskd_sb[:], emaskd_d[:])
                elif kind == "ident":
                    nc.sync.dma_start(ident_sb[:], ident_d[:])

            # RoPE: dest = psum*[cos;cos] + swap(psum)*[-sin;sin].
            # use_act: swap copies on ACT (fine in phase A); in pipeline slots
            # ACT is saturated by exps, so they go to Pool instead.
            def rope(psum, dest, s0, use_act=True):
                sw = rpool.tile([128, 512], f32, name="sw", tag="sw")
                if use_act:
                    nc.scalar.copy(sw[0:64, :], psum[64:128, :])
                    nc.scalar.copy(sw[64:128, :], psum[0:64, :])
                else:
                    # Pool cannot touch PSUM: split the swap DVE/ACT
                    nc.vector.tensor_copy(sw[0:64, :], psum[64:128, :])
                    nc.scalar.copy(sw[64:128, :], psum[0:64, :])
                t1 = rpool.tile([128, 512], f32, name="t1", tag="t1")
                nc.vector.tensor_tensor(t1[:], psum[:], cc_sb[:, s0:s0 + 512],
                                        op=mult)
                nc.gpsimd.tensor_tensor(sw[:], sw[:], ns_sb[:, s0:s0 + 512],
                                        op=mult)
                nc.vector.tensor_tensor(dest, t1[:], sw[:], op=add)

            if phases < 1:
                nul = const.tile([128, S], bf16, name="nul")
                nc.sync.dma_start(nul[:], xT[:, 0:1024])
                nc.sync.dma_start(outT[0:128, :], nul[:])
                return

            # ---------------- phase A ----------------
            # DMA schedule keyed on (sh, d) of the k stream; q0 lags QLAG
            # d-tiles and v lags VLAG so their weight DMAs can trail.
            QLAG = 4
            sched = {
                (0, 0): [("x", 0, 0, 1), ("wk", 0, 2)],
                (0, 1): [("x", 0, 1, 2), ("wk", 2, 4)],
                (0, 2): [("x", 0, 2, 3), ("wq0", 0, 4)],
                (0, 3): [("x", 0, 3, 4), ("wk", 4, 8)],
                (0, 4): [("x", 0, 4, 6), ("wq0", 4, 8)],
                (0, 6): [("x", 0, 6, 8), ("wk", 8, 16)],
                (0, 8): [("x", 0, 8, 10), ("wq0", 8, 16), ("wv", 0, 8)],
                (0, 10): [("x", 0, 10, 12), ("wk", 16, 24)],
                (0, 12): [("x", 0, 12, 14), ("wq0", 16, 24)],
                (0, 14): [("x", 0, 14, 16), ("wv", 8, 16)],
                (0, 16): [("x", 0, 16, 18), ("wk", 24, 32)],
                (0, 18): [("x", 0, 18, 20), ("wq0", 24, 32)],
                (0, 20): [("x", 0, 20, 22), ("wv", 16, 24)],
                (0, 22): [("x", 0, 22, 24), ("wv", 24, 32)],
                (0, 24): [("x", 0, 24, 26)],
                (0, 26): [("x", 0, 26, 28)],
                (0, 28): [("x", 0, 28, 30)],
                (0, 30): [("x", 0, 30, 32)],
                (0, 32): [("x", 1, 0, 2), ("cc",), ("ns",)],
                (0, 34): [("x", 1, 2, 4)],
                (0, 36): [("x", 1, 4, 6)],
                (0, 38): [("x", 1, 6, 8), ("emaskd",), ("ident",)],
                (1, 0): [("x", 1, 8, 10)],
                (1, 2): [("x", 1, 10, 12)],
                (1, 4): [("x", 1, 12, 14), ("wq1", 0, 16)],
                (1, 6): [("x", 1, 14, 16)],
                (1, 8): [("x", 1, 16, 18), ("wq1", 16, 32)],
                (1, 10): [("x", 1, 18, 20)],
                (1, 12): [("x", 1, 20, 22), ("wq2", 0, 16)],
                (1, 14): [("x", 1, 22, 24)],
                (1, 16): [("x", 1, 24, 26), ("wq2", 16, 32)],
                (1, 18): [("x", 1, 26, 28)],
                (1, 20): [("x", 1, 28, 30), ("wq3", 0, 16)],
                (1, 22): [("x", 1, 30, 32)],
                (1, 24): [("wq3", 16, 32)],
            }

            for sh in range(2):
                k_ps, q_ps, v_ps = P[3 * sh], P[1 + 3 * sh], P[2 + 3 * sh]
                s0 = 512 * sh
                for d in range(DT + VLAG):
                    for ev in sched.get((sh, d), []):
                        issue_dma(ev)
                    if d < DT:
                        nc.tensor.matmul(k_ps[:], wk_sb[:, 128 * d:128 * (d + 1)],
                                         xs(sh, d), start=(d == 0),
                                         stop=(d == DT - 1))
                    dq = d - QLAG
                    if 0 <= dq < DT:
                        nc.tensor.matmul(q_ps[:], wq_sb[0][:, 128 * dq:128 * (dq + 1)],
                                         xs(sh, dq), start=(dq == 0),
                                         stop=(dq == DT - 1))
                    dv = d - VLAG
                    if dv >= 0:
                        nc.tensor.matmul(v_ps[:], wv_sb[:, 128 * dv:128 * (dv + 1)],
                                         xs(sh, dv), start=(dv == 0),
                                         stop=(dv == DT - 1))
                if sh == 0:
                    rope(k_ps, k_rot[:, s0:s0 + 512], s0)
                    rope(q_ps, q_rot[0][:, s0:s0 + 512], s0)
                    nc.vector.tensor_copy(v_et[:, s0:s0 + 512], v_ps[:])
                else:
                    nc.vector.tensor_copy(v_et[:, s0:s0 + 512], v_ps[:])
                    rope(q_ps, q_rot[0][:, s0:s0 + 512], s0)
                    rope(k_ps, k_rot[:, s0:s0 + 512], s0)

            if phases < 2:
                nc.sync.dma_start(outT[0:128, 0:512], v_et[:, 0:1024].bitcast(bf16)[:, 0:512])
                return

            # sh0-half V transposes into P6/P7 (untouched so far). Their
            # v_et[:, 0:512] source was written early in phase-A sh1.
            for t in range(4):
                tr = P[6 + (t % 2)][:, 0:128]
                nc.tensor.matmul(tr, v_et[:, 128 * t:128 * (t + 1)], ident_sb[:],
                                 is_transpose=True)
                nc.vector.tensor_copy(v_te[:, 128 * t:128 * (t + 1)], tr)

            if phases == 2:
                nc.sync.dma_start(outT[0:128, :], v_te[:])
                return

            # ---------------- pipeline slots ----------------
            UNITS = [(0, 0), (1, 0), (2, 0), (3, 0), (0, 1), (1, 1), (2, 1),
                     (3, 1), (4, 0), (4, 1), (5, 0), (5, 1), (6, 0), (6, 1),
                     (7, 0), (7, 1)]
            PV_DELAY = 3

            def expm_of(h, t):
                return expmb[(8 * h + t) % 4]

            def emit_sc_exp(h, i, t, c):
                sc = P[6 + (i % 2)]
                lo, hi = 512 * c, 512 * (c + 1)
                dlo, dhi = 128 * t, 128 * (t + 1)
                nc.tensor.matmul(sc[:], k_rot[:, dlo:dhi], q_rot[h][:, lo:hi],
                                 start=True, stop=True)
                expm = expm_of(h, t)
                if dlo >= hi:
                    nc.scalar.activation(expm[:, lo:hi], sc[:], Exp,
                                         scale=inv_sqrt_hd)
                elif dhi <= lo:
                    nc.scalar.activation(expm[:, lo:hi], sc[:], Exp,
                                         scale=inv_sqrt_hd, bias=1.0)
                else:
                    nc.scalar.activation(expm[:, lo:hi], sc[:], Exp,
                                         scale=inv_sqrt_hd)
                    nc.gpsimd.tensor_tensor(
                        expm[:, dlo:dhi], expm[:, dlo:dhi],
                        emaskd_sb[:, 128 * t:128 * (t + 1)], op=mult)
                    if dhi < hi:
                        nc.gpsimd.tensor_scalar_mul(
                            expm[:, dhi:hi], expm[:, dhi:hi], float(np.e))

            def emit_pv(h, t, c):
                expm = expm_of(h, t)
                lo, hi = 512 * c, 512 * (c + 1)
                nc.tensor.matmul(P[c][:], v_te[:, 128 * t:128 * (t + 1)],
                                 expm[:, lo:hi], start=(t == 0),
                                 stop=(t == TT - 1))

            def emit_z(h, t, c):
                # single-shot per-(t, sb) partials at column 8t+sb: interleaved
                # open accumulation groups in one bank corrupt on HW, so the
                # t-sum happens later on DVE (3-step tree in tail 1)
                expm = expm_of(h, t)
                for sb in range(4 * c, 4 * c + 4):
                    nc.tensor.matmul(P[2][:, 8 * t + sb:8 * t + sb + 1],
                                     expm[:, 128 * sb:128 * (sb + 1)],
                                     onec_sb[:], start=True, stop=True,
                                     skip_group_check=True)

            def emit_qproj(hq, sh, d):
                bank = P[5 - sh]
                nc.tensor.matmul(bank[:], wq_sb[hq][:, 128 * d:128 * (d + 1)],
                                 xs(sh, d), start=(d == 0), stop=(d == DT - 1))
                if d == DT - 1:
                    rope(bank, q_rot[hq][:, 512 * sh:512 * sh + 512], 512 * sh,
                         use_act=False)

            def emit_tail2a(h, rz_cat):
                # scatter rz columns to partition-0 rows with 8 small
                # SBUF->SBUF DMAs (engines cannot read partitions 1..7, and
                # a PE [128,1]-transpose chain proved wrong on hardware)
                rz_bf = zpool.tile([128, 8], bf16, name="rz_bf", tag="rzbf",
                                   bufs=2)
                nc.scalar.copy(rz_bf[:], rz_cat[:])
                rts = []
                for sb in range(8):
                    rt = zpool.tile([1, 128], bf16, name=f"rt{sb}",
                                    tag=f"rt{sb}", bufs=1)
                    nc.sync.dma_start(rt[:], rz_bf[:, sb:sb + 1])
                    rts.append(rt)
                return rts

            def emit_tail2b(h, o_sb, rts):
                # broadcast rz across partitions into {P3 (c0), P2 (c1)},
                # then normalize the (already SBUF-decoupled) o into a[h].
                for sb in range(8):
                    nc.tensor.matmul(P[3 - (sb // 4)][:, 128 * (sb % 4):
                                                      128 * (sb % 4) + 128],
                                     oner_sb[:], rts[sb][:], start=True,
                                     stop=True)
                nc.vector.tensor_tensor(a[h][:, 0:512], o_sb[:, 0:512],
                                        P[3][:], op=mult)
                nc.vector.tensor_tensor(a[h][:, 512:1024], o_sb[:, 512:1024],
                                        P[2][:], op=mult)
                if phases == 3:
                    rz_sb = zpool.tile([128, S], bf16, name="rz_sb",
                                       tag="rzsb", bufs=4)
                    dbg_rz.append(rz_sb)
                    nc.scalar.copy(rz_sb[:, 0:512], P[3][:])
                    nc.vector.tensor_copy(rz_sb[:, 512:1024], P[2][:])

            NHEADS = NREP if phases >= 3 else 0
            dbg_rz = []
            pend_tail = None
            for h in range(NHEADS):
                qp = ([(h + 1, sh, d) for sh in range(2) for d in range(DT)]
                      if h + 1 < NREP else [])
                qi = 0
                if h < 2:
                    issue_dma(("wo", 2 * h, 0))
                    issue_dma(("wo", 2 * h + 1, 0))
                tail_a_i, tail_b_i = 2, 5
                tail_done = pend_tail is None
                pend_z = []
                fillers = ([(hh, c) for hh in range(3) for c in range(2)]
                           if h == NREP - 1 and phases >= 4 else [])
                units = UNITS
                for i, (t, c) in enumerate(units):
                    emit_sc_exp(h, i, t, c)
                    take = min(4, len(qp) - qi)
                    for _ in range(take):
                        emit_qproj(*qp[qi])
                        qi += 1
                    if h == 0 and 3 <= i <= 6:
                        # sh1-half V transposes (v_et source long ready)
                        t4 = i + 1  # 4..7
                        tr = P[3 + (i % 2)][:, 128:256]
                        nc.tensor.matmul(tr, v_et[:, 128 * t4:128 * (t4 + 1)],
                                         ident_sb[:], is_transpose=True)
                        nc.vector.tensor_copy(v_te[:, 128 * t4:128 * (t4 + 1)],
                                              tr)
                    if h == NREP - 1 and 8 <= i <= 13 and fillers:
                        # fill the qproj-less last slot: pre-accumulate
                        # phase-4 do=0 over heads 0..2 into {P4 (c0), P5 (c1)}
                        hh, cf = fillers.pop(0)
                        nc.tensor.matmul(P[4 + cf][:],
                                         wo_half[(hh, 0)][:, 0:128],
                                         a[hh][:, 512 * cf:512 * (cf + 1)],
                                         start=(hh == 0), stop=False,
                                         skip_group_check=True)
                    if i == tail_a_i and pend_tail is not None:
                        pend_rts = emit_tail2a(pend_tail[0], pend_tail[1])
                    if i == tail_b_i and pend_tail is not None:
                        emit_tail2b(pend_tail[0], pend_tail[2], pend_rts)
                        tail_done = True
                        pend_tail = None
                    if i >= PV_DELAY:
                        emit_pv(h, *units[i - PV_DELAY])
                        # z matmuls write P2, which tail2b(h-1)'s broadcast
                        # also writes: hold them until tail2b has been emitted
                        pend_z.append(units[i - PV_DELAY])
                        if tail_done:
                            while pend_z:
                                emit_z(h, *pend_z.pop(0))
                for j in range(PV_DELAY, 0, -1):
                    emit_pv(h, *units[len(units) - j])
                    pend_z.append(units[len(units) - j])
                while pend_z:
                    emit_z(h, *pend_z.pop(0))
                while qi < len(qp):
                    emit_qproj(*qp[qi])
                    qi += 1

                # tail part 1: tree-sum the z partials (cols 8t+sb), then
                # rz = 1/(z + 1024); free the o banks immediately by copying
                # o to SBUF
                zp = zpool.tile([128, 64], f32, name="zp", tag="zp", bufs=2)
                nc.vector.tensor_copy(zp[:], P[2][:, 0:64])
                nc.vector.tensor_tensor(zp[:, 0:32], zp[:, 0:32], zp[:, 32:64],
                                        op=add)
                nc.vector.tensor_tensor(zp[:, 0:16], zp[:, 0:16], zp[:, 16:32],
                                        op=add)
                z2 = zpool.tile([128, 8], f32, name="z2", tag="z2", bufs=2)
                rz_cat = zpool.tile([128, 8], f32, name="rz_cat", tag="rzc",
                                    bufs=2)
                nc.vector.tensor_tensor(z2[:], zp[:, 0:8], zp[:, 8:16], op=add)
                nc.vector.tensor_scalar_add(z2[:], z2[:], float(MAXSEQ - S))
                nc.vector.reciprocal(rz_cat[:], z2[:])
                o_sb = zpool.tile([128, S], bf16, name="o_sb", tag="osbuf",
                                  bufs=2)
                nc.scalar.copy(o_sb[:, 0:512], P[0][:])
                nc.vector.tensor_copy(o_sb[:, 512:1024], P[1][:])
                pend_tail = (h, rz_cat, o_sb)
                if h == NHEADS - 1:
                    pend_rts = emit_tail2a(h, rz_cat)
                    emit_tail2b(h, o_sb, pend_rts)
                    pend_tail = None

            if phases == 3:
                for h in range(NREP):
                    nc.sync.dma_start(outT[128 * h:128 * (h + 1), :], a[h][:])
                    nc.sync.dma_start(outT[128 * (4 + h):128 * (5 + h), :],
                                      dbg_rz[h][:])
                nc.sync.dma_start(outT[1024:1152, :], v_te[:])
                nc.sync.dma_start(outT[1152:1280, :], k_rot[:])
                nc.sync.dma_start(outT[1280:1408, :], q_rot[0][:])
                return

            # ---------------- phase 4: output projection ----------------
            for do in range(DT):
                if do == 0:
                    issue_dma(("wo", 0, 1))
                    issue_dma(("wo", 1, 1))
                if do == 4:
                    issue_dma(("wo", 2, 1))
                    issue_dma(("wo", 3, 1))
                op = [[P[4], P[5]], [P[6], P[7]], [P[0], P[1]]][do % 3]
                half, dl = do // 16, do % 16
                # do=0's hh=0..2 were pre-accumulated during the last slot
                for hh in ([3] if do == 0 else range(NREP)):
                    nc.tensor.matmul(op[0][:],
                                     wo_half[(hh, half)][:, 128 * dl:128 * (dl + 1)],
                                     a[hh][:, 0:512], start=(hh == 0 and do > 0),
                                     stop=(hh == NREP - 1),
                                     skip_group_check=True)
                    nc.tensor.matmul(op[1][:],
                                     wo_half[(hh, half)][:, 128 * dl:128 * (dl + 1)],
                                     a[hh][:, 512:1024],
                                     start=(hh == 0 and do > 0),
                                     stop=(hh == NREP - 1),
                                     skip_group_check=True)
                if do < DT - 1:
                    out_sb = opool.tile([128, S], bf16, name="out_sb", tag="osb",
                                        bufs=3)
                    nc.vector.tensor_copy(out_sb[:, 0:512], op[0][:])
                    nc.scalar.copy(out_sb[:, 512:1024], op[1][:])
                    nc.sync.dma_start(outT[128 * do:128 * (do + 1), :], out_sb[:])
                else:
                    # dedicated buffers so the final copies never wait on the
                    # out_sb/DMA rotation
                    oA = opool.tile([128, 512], bf16, name="oA", tag="oA", bufs=1)
                    oB = opool.tile([128, 512], bf16, name="oB", tag="oB", bufs=1)
                    nc.vector.tensor_copy(oA[:], op[0][:])
                    nc.sync.dma_start(outT[128 * do:128 * (do + 1), 0:512], oA[:])
                    nc.scalar.copy(oB[:], op[1][:])
                    nc.sync.dma_start(outT[128 * do:128 * (do + 1), 512:1024],
                                      oB[:])

        for _rep in range(repeat):
            _body()

    nc.compile()
    return nc


def kernel(**inputs):
    import ml_dtypes
    from concourse.bass_utils import run_bass_kernel_spmd

    bf = ml_dtypes.bfloat16
    x = np.asarray(inputs["x"], np.float32)                 # [1, S, D]
    cos = np.asarray(inputs["freqs_cos"], np.float32)       # [S, 64]
    sin = np.asarray(inputs["freqs_sin"], np.float32)       # [S, 64]
    wq = np.asarray(inputs["wq"], np.float32)               # [NH, HD, D]
    wk = np.asarray(inputs["wk"], np.float32)               # [NKV, HD, D]
    wv = np.asarray(inputs["wv"], np.float32)               # [NKV, HD, D]
    wo = np.asarray(inputs["wo"], np.float32)               # [D, D]
    input_pos = np.asarray(inputs["input_pos"]).astype(np.int64)  # [S]

    if "nc" not in _CACHE:
        _CACHE["nc"] = _build_nc()
    nc = _CACHE["nc"]

    perm = np.concatenate([np.arange(0, HD, 2), np.arange(1, HD, 2)])
    # x: [D, S] -> sh-major pack [128, 2*16384]
    xT = np.ascontiguousarray(
        x[0].T.reshape(DT, 128, 2, 512).transpose(1, 2, 0, 3)
        .reshape(128, 2 * DT * 512)).astype(bf)
    cc = np.ascontiguousarray(np.concatenate([cos.T, cos.T], 0)).astype(bf)
    ns = np.ascontiguousarray(np.concatenate([-sin.T, sin.T], 0)).astype(bf)
    # visibility adds +1 pre-exp where input_pos[t] <= input_pos[s]; for the
    # (spec-guaranteed) sorted arange fill only diagonal blocks are mixed.
    emaskd_t = np.empty((TT, 128, 128), np.float32)
    for t in range(TT):
        p = input_pos[128 * t:128 * (t + 1)]
        emaskd_t[t] = np.where(p[:, None] <= p[None, :], np.float32(np.e),
                               np.float32(1.0))
    emaskd = np.ascontiguousarray(
        emaskd_t.transpose(1, 0, 2).reshape(128, TT * 128)).astype(bf)
    ident = np.eye(128, dtype=np.float32)

    def pmajor(wT):
        # [D, 128e] -> [128p, DT*128e] partition-major
        return np.ascontiguousarray(
            wT.reshape(DT, 128, HD).transpose(1, 0, 2).reshape(128, DT * HD))

    in_maps = []
    for g in range(NCORES):
        wq_g = wq[NREP * g:NREP * (g + 1)][:, perm, :]       # [4, 128, D]
        wo_g = wo[:, NREP * HD * g:NREP * HD * (g + 1)].T    # [512, D]
        in_maps.append({
            "xT": xT,
            "wq_t": np.stack([pmajor(wq_g[j].T) for j in range(NREP)]).astype(bf),
            "wk_t": pmajor(wk[g][perm].T).astype(bf),
            "wv_t": pmajor(wv[g].T).astype(bf),
            "wo_t": np.ascontiguousarray(
                wo_g.reshape(NREP, 128, D).transpose(1, 0, 2)
                .reshape(128, NREP * D)).astype(bf),
            "cc": cc, "ns": ns, "emaskd": emaskd, "ident": ident,
        })

    res = run_bass_kernel_spmd(nc, in_maps, list(range(NCORES)))
    total = np.zeros((D, S), np.float32)
    for g in range(NCORES):
        total += np.asarray(res.results[g]["outT"], dtype=np.float32)
    return np.ascontiguousarray(total.T)[None]   # [1, S, D]


# revision 45
# speedup vs baseline: 1.1338x; 1.0470x over previous
"""Trainium2 Bass kernel for nn_AttentionSHA (dense transformer attention block).

Full inputs -> full output. Tensor-parallel over heads across 8 NeuronCores
(core g owns kv-head g and query heads 4g..4g+3; wo row-sharded), host-side
reduce of the 8 partial output projections.

v2 design (fused pipeline):
  - Everything DMA'd travels as bf16 (x, wq/wk/wv, wo, trig, emask, out).
    TRN2 matmul cost tracks the moving operand; bf16 moves at 1 cyc/row at
    any width. Host-side simulation of the bf16 input rounding measured
    6.2e-3 max-rel error (gate 2e-2; device fp32r noise adds ~5e-4).
  - Phase A projects k, q0 and (lagged by 8 d-tiles, so the wv DMA can
    trail) v for both halves; RoPE runs on ACT/DVE/Pool behind the PE.
  - Pipeline slots: slot h runs head h's attention while also projecting
    head h+1's q on the PE. Per-head attention alone is ACT-bound (16 exps
    at ~0.6us) once z is off the PE, so the q-projection keeps PE busy.
  - Softmax denominator z via stationary-side trick: matmuls with expm
    slices *stationary* and a [128,1] ones moving vector accumulate
    z[s-block] columns at ~1 cycle each (cost follows output moving size).
    rz = 1/(z+1024) is transposed (PE) and re-broadcast across partitions
    with eight [1,128]-moving matmuls against a [1,128] ones stationary.
  - The reference adds a 0/1 causal mask *before* softmax (no -inf) and
    softmaxes over MAXSEQ=2048 whose tail positions hold zero k/v, hence
    z = sum_t exp(sc)*m_t + 1024 with m_t = e if visible else 1; the
    e-factor folds into the Exp bias except on 128x128 diagonal blocks.
"""
import numpy as np
from contextlib import ExitStack

S = 1024
D = 4096
NH = 32
NKV = 8
HD = 128
NREP = NH // NKV          # 4
MAXSEQ = 2048
NCORES = 8
DT = D // 128             # 32 d-tiles
TT = S // 128             # 8 t-tiles
VLAG = 8                  # phase-A v-stream lag in d-tiles

_CACHE = {}


def _build_nc(phases=4, repeat=1):
    import concourse.bacc as bacc
    import concourse.mybir as mybir
    import concourse.tile as tile

    f32 = mybir.dt.float32
    f32r = mybir.dt.float32r
    bf16 = mybir.dt.bfloat16
    Exp = mybir.ActivationFunctionType.Exp
    mult = mybir.AluOpType.mult
    add = mybir.AluOpType.add

    nc = bacc.Bacc("TRN2", target_bir_lowering=False, debug=False,
                   num_devices=NCORES)

    # x packed sh-major: col = sh*16384 + d*512 + s_local
    xT = nc.dram_tensor("xT", [128, 2 * DT * 512], bf16, kind="ExternalInput")
    wq_t = nc.dram_tensor("wq_t", [NREP, 128, DT * HD], bf16, kind="ExternalInput")
    wk_t = nc.dram_tensor("wk_t", [128, DT * HD], bf16, kind="ExternalInput")
    wv_t = nc.dram_tensor("wv_t", [128, DT * HD], bf16, kind="ExternalInput")
    # wo packed per head then d-major: col = h*D + do*128 + e ... see host
    wo_t = nc.dram_tensor("wo_t", [128, NREP * D], bf16, kind="ExternalInput")
    cc_d = nc.dram_tensor("cc", [HD, S], bf16, kind="ExternalInput")
    ns_d = nc.dram_tensor("ns", [HD, S], bf16, kind="ExternalInput")
    emaskd_d = nc.dram_tensor("emaskd", [128, TT * 128], bf16, kind="ExternalInput")
    ident_d = nc.dram_tensor("ident", [128, 128], f32, kind="ExternalInput")
    outT = nc.dram_tensor("outT", [D, S], bf16, kind="ExternalOutput")

    inv_sqrt_hd = float(1.0 / np.sqrt(HD))

    with tile.TileContext(nc) as tc, ExitStack() as ctx:
        const = ctx.enter_context(tc.tile_pool(name="const", bufs=1))
        big = ctx.enter_context(tc.tile_pool(name="big", bufs=1))
        wts = ctx.enter_context(tc.tile_pool(name="wts", bufs=1))
        hs = ctx.enter_context(tc.tile_pool(name="hs", bufs=1))
        rpool = ctx.enter_context(tc.tile_pool(name="rpool", bufs=2))
        epool = ctx.enter_context(tc.tile_pool(name="epool", bufs=1))
        zpool = ctx.enter_context(tc.tile_pool(name="zpool", bufs=1))
        opool = ctx.enter_context(tc.tile_pool(name="opool", bufs=2))
        ps = ctx.enter_context(tc.tile_pool(name="ps", bufs=1, space="PSUM"))

        def _body():
            # ---- persistent PSUM banks, hand-assigned ----
            # phase A: sh0 {k:P0, q0:P1, v:P2}, sh1 {k:P3, q0:P4, v:P5}
            # slots:   sc {P6,P7}, o_ps {P0,P1}, z/zT {P2},
            #          qproj {sh0:P5, sh1:P4}, rz broadcast {P3, P2},
            #          slot-0 v-transposes {P3,P4}[:,128:256]
            # phase 4: op pairs {P6,P7} / {P0,P1}
            P = [ps.tile([128, 512], f32, name=f"bankP{i}", tag=f"bankP{i}")
                 for i in range(8)]

            # ---- constants ----
            cc_sb = const.tile([128, S], bf16)
            ns_sb = const.tile([128, S], bf16)
            ident_sb = const.tile([128, 128], f32)
            emaskd_sb = const.tile([128, TT * 128], bf16)
            onec_sb = const.tile([128, 1], bf16)     # z moving vector
            oner_sb = const.tile([1, 128], bf16)     # rz-broadcast stationary
            nc.gpsimd.memset(onec_sb[:], 1.0)
            nc.gpsimd.memset(oner_sb[:], 1.0)

            # ---- SBUF tensors ----
            x_sb = big.tile([128, 2 * DT * 512], bf16)
            wq_sb = [wts.tile([128, D], bf16, name=f"wq_sb{h}", tag=f"wq{h}")
                     for h in range(NREP)]
            wk_sb = wts.tile([128, D], bf16, tag="wk")
            wv_sb = wts.tile([128, D], bf16, tag="wv")
            # wo streamed in halves: tag wo{h} rotates 2 bufs of [128, 16*128]
            wo_half = {}

            q_rot = [hs.tile([128, S], bf16, name=f"q_rot{h}", tag=f"qr{h}")
                     for h in range(NREP)]
            k_rot = hs.tile([128, S], bf16, tag="kr")
            v_et = hs.tile([128, S], f32, tag="vet")     # [e, t] pre-transpose
            v_te = hs.tile([128, TT * 128], bf16, tag="vte")  # tile t: [t, e]
            a = [hs.tile([128, S], bf16, name=f"a{h}", tag=f"a{h}")
                 for h in range(NREP)]
            expmb = [epool.tile([128, S], bf16, name=f"expmb{i}", tag=f"eb{i}")
                     for i in range(4)]

            def xs(sh, d):
                c = sh * 16384 + d * 512
                return x_sb[:, c:c + 512]

            def issue_dma(ev):
                kind = ev[0]
                if kind == "x":
                    sh, d0, d1 = ev[1], ev[2], ev[3]
                    c0, c1 = sh * 16384 + d0 * 512, sh * 16384 + d1 * 512
                    nc.sync.dma_start(x_sb[:, c0:c1], xT[:, c0:c1])
                elif kind in ("wk", "wv") or kind.startswith("wq"):
                    w_sb, w_d = {"wk": (wk_sb, wk_t), "wv": (wv_sb, wv_t)}.get(
                        kind, (None, None))
                    if w_sb is None:
                        h = int(kind[2])
                        w_sb, w_d = wq_sb[h], wq_t[h]
                    d0, d1 = ev[1], ev[2]
                    nc.sync.dma_start(w_sb[:, 128 * d0:128 * d1],
                                      w_d[:, 128 * d0:128 * d1])
                elif kind == "wo":
                    h, half = ev[1], ev[2]
                    t_ = wts.tile([128, 16 * 128], bf16, name=f"wo{h}_{half}",
                                  tag=f"wo{h}", bufs=2)
                    wo_half[(h, half)] = t_
                    c0 = D * h + 2048 * half
                    nc.sync.dma_start(t_[:], wo_t[:, c0:c0 + 2048])
                elif kind == "cc":
                    nc.sync.dma_start(cc_sb[:], cc_d[:])
                elif kind == "ns":
                    nc.sync.dma_start(ns_sb[:], ns_d[:])
                elif kind == "emaskd":
                    nc.sync.dma_start(emaskd_sb[:], emaskd_d[:])
                elif kind == "ident":
                    nc.sync.dma_start(ident_sb[:], ident_d[:])

            # RoPE: dest = psum*[cos;cos] + swap(psum)*[-sin;sin].
            # use_act: swap copies on ACT (fine in phase A); in pipeline slots
            # ACT is saturated by exps, so they go to Pool instead.
            def rope(psum, dest, s0, use_act=True):
                sw = rpool.tile([128, 512], f32, name="sw", tag="sw")
                if use_act:
                    nc.scalar.copy(sw[0:64, :], psum[64:128, :])
                    nc.scalar.copy(sw[64:128, :], psum[0:64, :])
                else:
                    # Pool cannot touch PSUM: split the swap DVE/ACT
                    nc.vector.tensor_copy(sw[0:64, :], psum[64:128, :])
                    nc.scalar.copy(sw[64:128, :], psum[0:64, :])
                t1 = rpool.tile([128, 512], f32, name="t1", tag="t1")
                nc.vector.tensor_tensor(t1[:], psum[:], cc_sb[:, s0:s0 + 512],
                                        op=mult)
                nc.gpsimd.tensor_tensor(sw[:], sw[:], ns_sb[:, s0:s0 + 512],
                                        op=mult)
                nc.vector.tensor_tensor(dest, t1[:], sw[:], op=add)

            if phases < 1:
                nul = const.tile([128, S], bf16, name="nul")
                nc.sync.dma_start(nul[:], xT[:, 0:1024])
                nc.sync.dma_start(outT[0:128, :], nul[:])
                return

            # ---------------- phase A ----------------
            # DMA schedule keyed on (sh, d) of the k stream; q0 lags QLAG
            # d-tiles and v lags VLAG so their weight DMAs can trail.
            QLAG = 4
            sched = {
                (0, 0): [("x", 0, 0, 1), ("wk", 0, 2)],
                (0, 1): [("x", 0, 1, 2), ("wk", 2, 4)],
                (0, 2): [("x", 0, 2, 3), ("wq0", 0, 4)],
                (0, 3): [("x", 0, 3, 4), ("wk", 4, 8)],
                (0, 4): [("x", 0, 4, 6), ("wq0", 4, 8)],
                (0, 6): [("x", 0, 6, 8), ("wk", 8, 16)],
                (0, 8): [("x", 0, 8, 10), ("wq0", 8, 16), ("wv", 0, 8)],
                (0, 10): [("x", 0, 10, 12), ("wk", 16, 24)],
                (0, 12): [("x", 0, 12, 14), ("wq0", 16, 24)],
                (0, 14): [("x", 0, 14, 16), ("wv", 8, 16)],
                (0, 16): [("x", 0, 16, 18), ("wk", 24, 32)],
                (0, 18): [("x", 0, 18, 20), ("wq0", 24, 32)],
                (0, 20): [("x", 0, 20, 22), ("wv", 16, 24)],
                (0, 22): [("x", 0, 22, 24), ("wv", 24, 32)],
                (0, 24): [("x", 0, 24, 26)],
                (0, 26): [("x", 0, 26, 28)],
                (0, 28): [("x", 0, 28, 30)],
                (0, 30): [("x", 0, 30, 32)],
                (0, 32): [("x", 1, 0, 2), ("cc",), ("ns",)],
                (0, 34): [("x", 1, 2, 4)],
                (0, 36): [("x", 1, 4, 6)],
                (0, 38): [("x", 1, 6, 8), ("emaskd",), ("ident",)],
                (1, 0): [("x", 1, 8, 10)],
                (1, 2): [("x", 1, 10, 12)],
                (1, 4): [("x", 1, 12, 14), ("wq1", 0, 16)],
                (1, 6): [("x", 1, 14, 16)],
                (1, 8): [("x", 1, 16, 18), ("wq1", 16, 32)],
                (1, 10): [("x", 1, 18, 20)],
                (1, 12): [("x", 1, 20, 22), ("wq2", 0, 16)],
                (1, 14): [("x", 1, 22, 24)],
                (1, 16): [("x", 1, 24, 26), ("wq2", 16, 32)],
                (1, 18): [("x", 1, 26, 28)],
                (1, 20): [("x", 1, 28, 30), ("wq3", 0, 16)],
                (1, 22): [("x", 1, 30, 32)],
                (1, 24): [("wq3", 16, 32)],
            }

            for sh in range(2):
                k_ps, q_ps, v_ps = P[3 * sh], P[1 + 3 * sh], P[2 + 3 * sh]
                s0 = 512 * sh
                for d in range(DT + VLAG):
                    for ev in sched.get((sh, d), []):
                        issue_dma(ev)
                    if d < DT:
                        nc.tensor.matmul(k_ps[:], wk_sb[:, 128 * d:128 * (d + 1)],
                                         xs(sh, d), start=(d == 0),
                                         stop=(d == DT - 1))
                    dq = d - QLAG
                    if 0 <= dq < DT:
                        nc.tensor.matmul(q_ps[:], wq_sb[0][:, 128 * dq:128 * (dq + 1)],
                                         xs(sh, dq), start=(dq == 0),
                                         stop=(dq == DT - 1))
                    dv = d - VLAG
                    if dv >= 0:
                        nc.tensor.matmul(v_ps[:], wv_sb[:, 128 * dv:128 * (dv + 1)],
                                         xs(sh, dv), start=(dv == 0),
                                         stop=(dv == DT - 1))
                if sh == 0:
                    rope(k_ps, k_rot[:, s0:s0 + 512], s0)
                    rope(q_ps, q_rot[0][:, s0:s0 + 512], s0)
                    nc.vector.tensor_copy(v_et[:, s0:s0 + 512], v_ps[:])
                else:
                    nc.vector.tensor_copy(v_et[:, s0:s0 + 512], v_ps[:])
                    rope(q_ps, q_rot[0][:, s0:s0 + 512], s0)
                    rope(k_ps, k_rot[:, s0:s0 + 512], s0)

            if phases < 2:
                nc.sync.dma_start(outT[0:128, 0:512], v_et[:, 0:1024].bitcast(bf16)[:, 0:512])
                return

            # sh0-half V transposes into P6/P7 (untouched so far). Their
            # v_et[:, 0:512] source was written early in phase-A sh1.
            for t in range(4):
                tr = P[6 + (t % 2)][:, 0:128]
                nc.tensor.matmul(tr, v_et[:, 128 * t:128 * (t + 1)], ident_sb[:],
                                 is_transpose=True)
                nc.vector.tensor_copy(v_te[:, 128 * t:128 * (t + 1)], tr)

            if phases == 2:
                nc.sync.dma_start(outT[0:128, :], v_te[:])
                return

            # ---------------- pipeline slots ----------------
            UNITS = [(0, 0), (1, 0), (2, 0), (3, 0), (0, 1), (1, 1), (2, 1),
                     (3, 1), (4, 0), (4, 1), (5, 0), (5, 1), (6, 0), (6, 1),
                     (7, 0), (7, 1)]
            PV_DELAY = 3

            def expm_of(h, t):
                return expmb[(8 * h + t) % 4]

            def emit_sc_exp(h, i, t, c):
                sc = P[6 + (i % 2)]
                lo, hi = 512 * c, 512 * (c + 1)
                dlo, dhi = 128 * t, 128 * (t + 1)
                nc.tensor.matmul(sc[:], k_rot[:, dlo:dhi], q_rot[h][:, lo:hi],
                                 start=True, stop=True)
                expm = expm_of(h, t)
                if dlo >= hi:
                    nc.scalar.activation(expm[:, lo:hi], sc[:], Exp,
                                         scale=inv_sqrt_hd)
                elif dhi <= lo:
                    nc.scalar.activation(expm[:, lo:hi], sc[:], Exp,
                                         scale=inv_sqrt_hd, bias=1.0)
                else:
                    nc.scalar.activation(expm[:, lo:hi], sc[:], Exp,
                                         scale=inv_sqrt_hd)
                    nc.gpsimd.tensor_tensor(
                        expm[:, dlo:dhi], expm[:, dlo:dhi],
                        emaskd_sb[:, 128 * t:128 * (t + 1)], op=mult)
                    if dhi < hi:
                        nc.gpsimd.tensor_scalar_mul(
                            expm[:, dhi:hi], expm[:, dhi:hi], float(np.e))

            def emit_pv(h, t, c):
                expm = expm_of(h, t)
                lo, hi = 512 * c, 512 * (c + 1)
                nc.tensor.matmul(P[c][:], v_te[:, 128 * t:128 * (t + 1)],
                                 expm[:, lo:hi], start=(t == 0),
                                 stop=(t == TT - 1))

            def emit_z(h, t, c):
                # single-shot per-(t, sb) partials at column 8t+sb: interleaved
                # open accumulation groups in one bank corrupt on HW, so the
                # t-sum happens later on DVE (3-step tree in tail 1)
                expm = expm_of(h, t)
                for sb in range(4 * c, 4 * c + 4):
                    nc.tensor.matmul(P[2][:, 8 * t + sb:8 * t + sb + 1],
                                     expm[:, 128 * sb:128 * (sb + 1)],
                                     onec_sb[:], start=True, stop=True,
                                     skip_group_check=True)

            def emit_qproj(hq, sh, d):
                bank = P[5 - sh]
                nc.tensor.matmul(bank[:], wq_sb[hq][:, 128 * d:128 * (d + 1)],
                                 xs(sh, d), start=(d == 0), stop=(d == DT - 1))
                if d == DT - 1:
                    rope(bank, q_rot[hq][:, 512 * sh:512 * sh + 512], 512 * sh,
                         use_act=False)

            def emit_tail2a(h, rz_cat):
                # scatter rz columns to partition-0 rows with 8 small
                # SBUF->SBUF DMAs (engines cannot read partitions 1..7, and
                # a PE [128,1]-transpose chain proved wrong on hardware)
                rz_bf = zpool.tile([128, 8], bf16, name="rz_bf", tag="rzbf",
                                   bufs=2)
                nc.scalar.copy(rz_bf[:], rz_cat[:])
                # one gather DMA (partition-outer order: dst[8p+sb]=rz_bf[p,sb])
                # instead of 8 per-column DMAs, which cost ~700ns of queue each
                rts_all = zpool.tile([1, 1024], bf16, name="rts_all",
                                     tag="rtsall", bufs=2)
                nc.sync.dma_start(rts_all[:], rz_bf[:])
                return rts_all

            def emit_tail2b(h, o_sb, rts):
                # broadcast rz across partitions into {P3 (c0), P2 (c1)},
                # then normalize the (already SBUF-decoupled) o into a[h].
                for sb in range(8):
                    nc.tensor.matmul(P[3 - (sb // 4)][:, 128 * (sb % 4):
                                                      128 * (sb % 4) + 128],
                                     oner_sb[:], rts[0:1, sb:1024:8],
                                     start=True, stop=True)
                nc.vector.tensor_tensor(a[h][:, 0:512], o_sb[:, 0:512],
                                        P[3][:], op=mult)
                nc.vector.tensor_tensor(a[h][:, 512:1024], o_sb[:, 512:1024],
                                        P[2][:], op=mult)
                if phases == 3:
                    rz_sb = zpool.tile([128, S], bf16, name="rz_sb",
                                       tag="rzsb", bufs=4)
                    dbg_rz.append(rz_sb)
                    nc.scalar.copy(rz_sb[:, 0:512], P[3][:])
                    nc.vector.tensor_copy(rz_sb[:, 512:1024], P[2][:])

            NHEADS = NREP if phases >= 3 else 0
            dbg_rz = []
            pend_tail = None
            for h in range(NHEADS):
                qp = ([(h + 1, sh, d) for sh in range(2) for d in range(DT)]
                      if h + 1 < NREP else [])
                qi = 0
                if h < 2:
                    issue_dma(("wo", 2 * h, 0))
                    issue_dma(("wo", 2 * h + 1, 0))
                tail_a_i, tail_b_i = 2, 5
                tail_done = pend_tail is None
                pend_z = []
                fillers = ([(hh, c) for hh in range(3) for c in range(2)]
                           if h == NREP - 1 and phases >= 4 else [])
                units = UNITS
                for i, (t, c) in enumerate(units):
                    emit_sc_exp(h, i, t, c)
                    take = min(4, len(qp) - qi)
                    for _ in range(take):
                        emit_qproj(*qp[qi])
                        qi += 1
                    if h == 0 and 3 <= i <= 6:
                        # sh1-half V transposes (v_et source long ready)
                        t4 = i + 1  # 4..7
                        tr = P[3 + (i % 2)][:, 128:256]
                        nc.tensor.matmul(tr, v_et[:, 128 * t4:128 * (t4 + 1)],
                                         ident_sb[:], is_transpose=True)
                        nc.vector.tensor_copy(v_te[:, 128 * t4:128 * (t4 + 1)],
                                              tr)
                    if h == NREP - 1 and 8 <= i <= 13 and fillers:
                        # fill the qproj-less last slot: pre-accumulate
                        # phase-4 do=0 over heads 0..2 into {P4 (c0), P5 (c1)}
                        hh, cf = fillers.pop(0)
                        nc.tensor.matmul(P[4 + cf][:],
                                         wo_half[(hh, 0)][:, 0:128],
                                         a[hh][:, 512 * cf:512 * (cf + 1)],
                                         start=(hh == 0), stop=False,
                                         skip_group_check=True)
                    if i == tail_a_i and pend_tail is not None:
                        pend_rts = emit_tail2a(pend_tail[0], pend_tail[1])
                    if i == tail_b_i and pend_tail is not None:
                        emit_tail2b(pend_tail[0], pend_tail[2], pend_rts)
                        tail_done = True
                        pend_tail = None
                    if i >= PV_DELAY:
                        emit_pv(h, *units[i - PV_DELAY])
                        # z matmuls write P2, which tail2b(h-1)'s broadcast
                        # also writes: hold them until tail2b has been emitted
                        pend_z.append(units[i - PV_DELAY])
                        if tail_done:
                            while pend_z:
                                emit_z(h, *pend_z.pop(0))
                for j in range(PV_DELAY, 0, -1):
                    emit_pv(h, *units[len(units) - j])
                    pend_z.append(units[len(units) - j])
                while pend_z:
                    emit_z(h, *pend_z.pop(0))
                while qi < len(qp):
                    emit_qproj(*qp[qi])
                    qi += 1

                # tail part 1: tree-sum the z partials (cols 8t+sb), then
                # rz = 1/(z + 1024); free the o banks immediately by copying
                # o to SBUF
                zp = zpool.tile([128, 64], f32, name="zp", tag="zp", bufs=2)
                nc.vector.tensor_copy(zp[:], P[2][:, 0:64])
                nc.vector.tensor_tensor(zp[:, 0:32], zp[:, 0:32], zp[:, 32:64],
                                        op=add)
                nc.vector.tensor_tensor(zp[:, 0:16], zp[:, 0:16], zp[:, 16:32],
                                        op=add)
                z2 = zpool.tile([128, 8], f32, name="z2", tag="z2", bufs=2)
                rz_cat = zpool.tile([128, 8], f32, name="rz_cat", tag="rzc",
                                    bufs=2)
                nc.vector.tensor_tensor(z2[:], zp[:, 0:8], zp[:, 8:16], op=add)
                nc.vector.tensor_scalar_add(z2[:], z2[:], float(MAXSEQ - S))
                nc.vector.reciprocal(rz_cat[:], z2[:])
                o_sb = zpool.tile([128, S], bf16, name="o_sb", tag="osbuf",
                                  bufs=2)
                nc.scalar.copy(o_sb[:, 0:512], P[0][:])
                nc.vector.tensor_copy(o_sb[:, 512:1024], P[1][:])
                pend_tail = (h, rz_cat, o_sb)
                if h == NHEADS - 1:
                    # issue the rz scatter DMAs now, but hold the bc matmuls:
                    # blocked matmuls fill the PE's 4-deep wait queue, so they
                    # must sit behind runnable phase-4 work (emitted below)
                    pend_rts = emit_tail2a(h, rz_cat)
                    last_tail = (h, o_sb, pend_rts)
                    pend_tail = None
                    if phases == 3:
                        emit_tail2b(*last_tail)

            if phases == 3:
                for h in range(NREP):
                    nc.sync.dma_start(outT[128 * h:128 * (h + 1), :], a[h][:])
                    nc.sync.dma_start(outT[128 * (4 + h):128 * (5 + h), :],
                                      dbg_rz[h][:])
                nc.sync.dma_start(outT[1024:1152, :], v_te[:])
                nc.sync.dma_start(outT[1152:1280, :], k_rot[:])
                nc.sync.dma_start(outT[1280:1408, :], q_rot[0][:])
                return

            # ---------------- phase 4: output projection ----------------
            PAIRS = [[P[4], P[5]], [P[6], P[7]], [P[0], P[1]]]

            def wo_mms(do, hh, start, stop):
                op = PAIRS[do % 3]
                half, dl = do // 16, do % 16
                w = wo_half[(hh, half)][:, 128 * dl:128 * (dl + 1)]
                for c_ in range(2):
                    nc.tensor.matmul(op[c_][:], w,
                                     a[hh][:, 512 * c_:512 * (c_ + 1)],
                                     start=start, stop=stop,
                                     skip_group_check=True)

            def wo_out(do):
                op = PAIRS[do % 3]
                if do < DT - 1:
                    out_sb = opool.tile([128, S], bf16, name="out_sb",
                                        tag="osb", bufs=3)
                    nc.vector.tensor_copy(out_sb[:, 0:512], op[0][:])
                    nc.scalar.copy(out_sb[:, 512:1024], op[1][:])
                    nc.sync.dma_start(outT[128 * do:128 * (do + 1), :],
                                      out_sb[:])
                else:
                    # dedicated buffers so the final copies never wait on the
                    # out_sb/DMA rotation
                    oA = opool.tile([128, 512], bf16, name="oA", tag="oA",
                                    bufs=1)
                    oB = opool.tile([128, 512], bf16, name="oB", tag="oB",
                                    bufs=1)
                    nc.vector.tensor_copy(oA[:], op[0][:])
                    nc.sync.dma_start(outT[128 * do:128 * (do + 1), 0:512],
                                      oA[:])
                    nc.scalar.copy(oB[:], op[1][:])
                    nc.sync.dma_start(outT[128 * do:128 * (do + 1), 512:1024],
                                      oB[:])

            issue_dma(("wo", 0, 1))
            issue_dma(("wo", 1, 1))
            # a[3] arrives late (head 3's rz scatter DMAs trail the last
            # slot), so run do 1-2 over heads 0-2 first; do 0's heads 0-2
            # were pre-accumulated as last-slot fillers.
            for do in (1, 2):
                for hh in range(3):
                    wo_mms(do, hh, start=(hh == 0), stop=False)
            emit_tail2b(*last_tail)
            for do in (0, 1, 2):
                wo_mms(do, 3, start=False, stop=True)
                wo_out(do)
            for do in range(3, DT):
                if do == 4:
                    issue_dma(("wo", 2, 1))
                    issue_dma(("wo", 3, 1))
                for hh in range(NREP):
                    wo_mms(do, hh, start=(hh == 0), stop=(hh == NREP - 1))
                wo_out(do)

        for _rep in range(repeat):
            _body()

    nc.compile()
    return nc


def kernel(**inputs):
    import ml_dtypes
    from concourse.bass_utils import run_bass_kernel_spmd

    bf = ml_dtypes.bfloat16
    x = np.asarray(inputs["x"], np.float32)                 # [1, S, D]
    cos = np.asarray(inputs["freqs_cos"], np.float32)       # [S, 64]
    sin = np.asarray(inputs["freqs_sin"], np.float32)       # [S, 64]
    wq = np.asarray(inputs["wq"], np.float32)               # [NH, HD, D]
    wk = np.asarray(inputs["wk"], np.float32)               # [NKV, HD, D]
    wv = np.asarray(inputs["wv"], np.float32)               # [NKV, HD, D]
    wo = np.asarray(inputs["wo"], np.float32)               # [D, D]
    input_pos = np.asarray(inputs["input_pos"]).astype(np.int64)  # [S]

    if "nc" not in _CACHE:
        _CACHE["nc"] = _build_nc()
    nc = _CACHE["nc"]

    perm = np.concatenate([np.arange(0, HD, 2), np.arange(1, HD, 2)])
    # x: [D, S] -> sh-major pack [128, 2*16384]
    xT = np.ascontiguousarray(
        x[0].T.reshape(DT, 128, 2, 512).transpose(1, 2, 0, 3)
        .reshape(128, 2 * DT * 512)).astype(bf)
    cc = np.ascontiguousarray(np.concatenate([cos.T, cos.T], 0)).astype(bf)
    ns = np.ascontiguousarray(np.concatenate([-sin.T, sin.T], 0)).astype(bf)
    # visibility adds +1 pre-exp where input_pos[t] <= input_pos[s]; for the
    # (spec-guaranteed) sorted arange fill only diagonal blocks are mixed.
    emaskd_t = np.empty((TT, 128, 128), np.float32)
    for t in range(TT):
        p = input_pos[128 * t:128 * (t + 1)]
        emaskd_t[t] = np.where(p[:, None] <= p[None, :], np.float32(np.e),
                               np.float32(1.0))
    emaskd = np.ascontiguousarray(
        emaskd_t.transpose(1, 0, 2).reshape(128, TT * 128)).astype(bf)
    ident = np.eye(128, dtype=np.float32)

    def pmajor(wT):
        # [D, 128e] -> [128p, DT*128e] partition-major
        return np.ascontiguousarray(
            wT.reshape(DT, 128, HD).transpose(1, 0, 2).reshape(128, DT * HD))

    in_maps = []
    for g in range(NCORES):
        wq_g = wq[NREP * g:NREP * (g + 1)][:, perm, :]       # [4, 128, D]
        wo_g = wo[:, NREP * HD * g:NREP * HD * (g + 1)].T    # [512, D]
        in_maps.append({
            "xT": xT,
            "wq_t": np.stack([pmajor(wq_g[j].T) for j in range(NREP)]).astype(bf),
            "wk_t": pmajor(wk[g][perm].T).astype(bf),
            "wv_t": pmajor(wv[g].T).astype(bf),
            "wo_t": np.ascontiguousarray(
                wo_g.reshape(NREP, 128, D).transpose(1, 0, 2)
                .reshape(128, NREP * D)).astype(bf),
            "cc": cc, "ns": ns, "emaskd": emaskd, "ident": ident,
        })

    res = run_bass_kernel_spmd(nc, in_maps, list(range(NCORES)))
    total = np.zeros((D, S), np.float32)
    for g in range(NCORES):
        total += np.asarray(res.results[g]["outT"], dtype=np.float32)
    return np.ascontiguousarray(total.T)[None]   # [1, S, D]


# revision 46
# speedup vs baseline: 1.1410x; 1.0063x over previous
"""Trainium2 Bass kernel for nn_AttentionSHA (dense transformer attention block).

Full inputs -> full output. Tensor-parallel over heads across 8 NeuronCores
(core g owns kv-head g and query heads 4g..4g+3; wo row-sharded), host-side
reduce of the 8 partial output projections.

v2 design (fused pipeline):
  - Everything DMA'd travels as bf16 (x, wq/wk/wv, wo, trig, emask, out).
    TRN2 matmul cost tracks the moving operand; bf16 moves at 1 cyc/row at
    any width. Host-side simulation of the bf16 input rounding measured
    6.2e-3 max-rel error (gate 2e-2; device fp32r noise adds ~5e-4).
  - Phase A projects k, q0 and (lagged by 8 d-tiles, so the wv DMA can
    trail) v for both halves; RoPE runs on ACT/DVE/Pool behind the PE.
  - Pipeline slots: slot h runs head h's attention while also projecting
    head h+1's q on the PE. Per-head attention alone is ACT-bound (16 exps
    at ~0.6us) once z is off the PE, so the q-projection keeps PE busy.
  - Softmax denominator z via stationary-side trick: matmuls with expm
    slices *stationary* and a [128,1] ones moving vector accumulate
    z[s-block] columns at ~1 cycle each (cost follows output moving size).
    rz = 1/(z+1024) is transposed (PE) and re-broadcast across partitions
    with eight [1,128]-moving matmuls against a [1,128] ones stationary.
  - The reference adds a 0/1 causal mask *before* softmax (no -inf) and
    softmaxes over MAXSEQ=2048 whose tail positions hold zero k/v, hence
    z = sum_t exp(sc)*m_t + 1024 with m_t = e if visible else 1; the
    e-factor folds into the Exp bias except on 128x128 diagonal blocks.
"""
import numpy as np
from contextlib import ExitStack

S = 1024
D = 4096
NH = 32
NKV = 8
HD = 128
NREP = NH // NKV          # 4
MAXSEQ = 2048
NCORES = 8
DT = D // 128             # 32 d-tiles
TT = S // 128             # 8 t-tiles
VLAG = 8                  # phase-A v-stream lag in d-tiles

_CACHE = {}


def _build_nc(phases=4, repeat=1):
    import concourse.bacc as bacc
    import concourse.mybir as mybir
    import concourse.tile as tile

    f32 = mybir.dt.float32
    f32r = mybir.dt.float32r
    bf16 = mybir.dt.bfloat16
    Exp = mybir.ActivationFunctionType.Exp
    mult = mybir.AluOpType.mult
    add = mybir.AluOpType.add

    nc = bacc.Bacc("TRN2", target_bir_lowering=False, debug=False,
                   num_devices=NCORES)

    # x packed sh-major: col = sh*16384 + d*512 + s_local
    xT = nc.dram_tensor("xT", [128, 2 * DT * 512], bf16, kind="ExternalInput")
    wq_t = nc.dram_tensor("wq_t", [NREP, 128, DT * HD], bf16, kind="ExternalInput")
    wk_t = nc.dram_tensor("wk_t", [128, DT * HD], bf16, kind="ExternalInput")
    wv_t = nc.dram_tensor("wv_t", [128, DT * HD], bf16, kind="ExternalInput")
    # wo packed per head then d-major: col = h*D + do*128 + e ... see host
    wo_t = nc.dram_tensor("wo_t", [128, NREP * D], bf16, kind="ExternalInput")
    cc_d = nc.dram_tensor("cc", [HD, S], bf16, kind="ExternalInput")
    ns_d = nc.dram_tensor("ns", [HD, S], bf16, kind="ExternalInput")
    emaskd_d = nc.dram_tensor("emaskd", [128, TT * 128], bf16, kind="ExternalInput")
    ident_d = nc.dram_tensor("ident", [128, 128], f32, kind="ExternalInput")
    outT = nc.dram_tensor("outT", [D, S], bf16, kind="ExternalOutput")

    inv_sqrt_hd = float(1.0 / np.sqrt(HD))

    with tile.TileContext(nc) as tc, ExitStack() as ctx:
        const = ctx.enter_context(tc.tile_pool(name="const", bufs=1))
        big = ctx.enter_context(tc.tile_pool(name="big", bufs=1))
        wts = ctx.enter_context(tc.tile_pool(name="wts", bufs=1))
        hs = ctx.enter_context(tc.tile_pool(name="hs", bufs=1))
        rpool = ctx.enter_context(tc.tile_pool(name="rpool", bufs=2))
        epool = ctx.enter_context(tc.tile_pool(name="epool", bufs=1))
        zpool = ctx.enter_context(tc.tile_pool(name="zpool", bufs=1))
        opool = ctx.enter_context(tc.tile_pool(name="opool", bufs=2))
        ps = ctx.enter_context(tc.tile_pool(name="ps", bufs=1, space="PSUM"))

        def _body():
            # ---- persistent PSUM banks, hand-assigned ----
            # phase A: sh0 {k:P0, q0:P1, v:P2}, sh1 {k:P3, q0:P4, v:P5}
            # slots:   sc {P6,P7}, o_ps {P0,P1}, z/zT {P2},
            #          qproj {sh0:P5, sh1:P4}, rz broadcast {P3, P2},
            #          slot-0 v-transposes {P3,P4}[:,128:256]
            # phase 4: op pairs {P6,P7} / {P0,P1}
            P = [ps.tile([128, 512], f32, name=f"bankP{i}", tag=f"bankP{i}")
                 for i in range(8)]

            # ---- constants ----
            cc_sb = const.tile([128, S], bf16)
            ns_sb = const.tile([128, S], bf16)
            ident_sb = const.tile([128, 128], f32)
            emaskd_sb = const.tile([128, TT * 128], bf16)
            onec_sb = const.tile([128, 1], bf16)     # z moving vector
            oner_sb = const.tile([1, 128], bf16)     # rz-broadcast stationary
            nc.gpsimd.memset(onec_sb[:], 1.0)
            nc.gpsimd.memset(oner_sb[:], 1.0)

            # ---- SBUF tensors ----
            x_sb = big.tile([128, 2 * DT * 512], bf16)
            wq_sb = [wts.tile([128, D], bf16, name=f"wq_sb{h}", tag=f"wq{h}")
                     for h in range(NREP)]
            wk_sb = wts.tile([128, D], bf16, tag="wk")
            wv_sb = wts.tile([128, D], bf16, tag="wv")
            # wo streamed in halves: tag wo{h} rotates 2 bufs of [128, 16*128]
            wo_half = {}

            q_rot = [hs.tile([128, S], bf16, name=f"q_rot{h}", tag=f"qr{h}")
                     for h in range(NREP)]
            k_rot = hs.tile([128, S], bf16, tag="kr")
            v_et = hs.tile([128, S], f32, tag="vet")     # [e, t] pre-transpose
            v_te = hs.tile([128, TT * 128], bf16, tag="vte")  # tile t: [t, e]
            a = [hs.tile([128, S], bf16, name=f"a{h}", tag=f"a{h}")
                 for h in range(NREP)]
            expmb = [epool.tile([128, S], bf16, name=f"expmb{i}", tag=f"eb{i}")
                     for i in range(4)]

            def xs(sh, d):
                c = sh * 16384 + d * 512
                return x_sb[:, c:c + 512]

            def issue_dma(ev):
                kind = ev[0]
                if kind == "x":
                    sh, d0, d1 = ev[1], ev[2], ev[3]
                    c0, c1 = sh * 16384 + d0 * 512, sh * 16384 + d1 * 512
                    nc.sync.dma_start(x_sb[:, c0:c1], xT[:, c0:c1])
                elif kind in ("wk", "wv") or kind.startswith("wq"):
                    w_sb, w_d = {"wk": (wk_sb, wk_t), "wv": (wv_sb, wv_t)}.get(
                        kind, (None, None))
                    if w_sb is None:
                        h = int(kind[2])
                        w_sb, w_d = wq_sb[h], wq_t[h]
                    d0, d1 = ev[1], ev[2]
                    nc.sync.dma_start(w_sb[:, 128 * d0:128 * d1],
                                      w_d[:, 128 * d0:128 * d1])
                elif kind == "wo":
                    h, half = ev[1], ev[2]
                    t_ = wts.tile([128, 16 * 128], bf16, name=f"wo{h}_{half}",
                                  tag=f"wo{h}", bufs=2)
                    wo_half[(h, half)] = t_
                    c0 = D * h + 2048 * half
                    nc.sync.dma_start(t_[:], wo_t[:, c0:c0 + 2048])
                elif kind == "cc":
                    nc.sync.dma_start(cc_sb[:], cc_d[:])
                elif kind == "ns":
                    nc.sync.dma_start(ns_sb[:], ns_d[:])
                elif kind == "emaskd":
                    nc.sync.dma_start(emaskd_sb[:], emaskd_d[:])
                elif kind == "ident":
                    nc.sync.dma_start(ident_sb[:], ident_d[:])

            # RoPE: dest = psum*[cos;cos] + swap(psum)*[-sin;sin].
            # use_act: swap copies on ACT (fine in phase A); in pipeline slots
            # ACT is saturated by exps, so they go to Pool instead.
            def rope(psum, dest, s0, use_act=True):
                sw = rpool.tile([128, 512], f32, name="sw", tag="sw")
                if use_act:
                    nc.scalar.copy(sw[0:64, :], psum[64:128, :])
                    nc.scalar.copy(sw[64:128, :], psum[0:64, :])
                else:
                    # Pool cannot touch PSUM: split the swap DVE/ACT
                    nc.vector.tensor_copy(sw[0:64, :], psum[64:128, :])
                    nc.scalar.copy(sw[64:128, :], psum[0:64, :])
                t1 = rpool.tile([128, 512], f32, name="t1", tag="t1")
                nc.vector.tensor_tensor(t1[:], psum[:], cc_sb[:, s0:s0 + 512],
                                        op=mult)
                nc.gpsimd.tensor_tensor(sw[:], sw[:], ns_sb[:, s0:s0 + 512],
                                        op=mult)
                nc.vector.tensor_tensor(dest, t1[:], sw[:], op=add)

            if phases < 1:
                nul = const.tile([128, S], bf16, name="nul")
                nc.sync.dma_start(nul[:], xT[:, 0:1024])
                nc.sync.dma_start(outT[0:128, :], nul[:])
                return

            # ---------------- phase A ----------------
            # DMA schedule keyed on (sh, d) of the k stream; q0 lags QLAG
            # d-tiles and v lags VLAG so their weight DMAs can trail.
            QLAG = 4
            sched = {
                (0, 0): [("x", 0, 0, 2), ("wk", 0, 2)],
                (0, 1): [("x", 0, 2, 4), ("wk", 2, 4)],
                (0, 2): [("wq0", 0, 4)],
                (0, 3): [("x", 0, 4, 6), ("wk", 4, 8)],
                (0, 4): [("wq0", 4, 8)],
                (0, 6): [("x", 0, 6, 8), ("wk", 8, 16)],
                (0, 8): [("x", 0, 8, 10), ("wq0", 8, 16), ("wv", 0, 8)],
                (0, 10): [("x", 0, 10, 12), ("wk", 16, 24)],
                (0, 12): [("x", 0, 12, 14), ("wq0", 16, 24)],
                (0, 14): [("x", 0, 14, 16), ("wv", 8, 16)],
                (0, 16): [("x", 0, 16, 18), ("wk", 24, 32)],
                (0, 18): [("x", 0, 18, 20), ("wq0", 24, 32)],
                (0, 20): [("x", 0, 20, 22), ("wv", 16, 24)],
                (0, 22): [("x", 0, 22, 24), ("wv", 24, 32)],
                (0, 24): [("x", 0, 24, 26)],
                (0, 26): [("x", 0, 26, 28)],
                (0, 28): [("x", 0, 28, 30)],
                (0, 30): [("x", 0, 30, 32)],
                (0, 32): [("x", 1, 0, 2), ("cc",), ("ns",)],
                (0, 34): [("x", 1, 2, 4)],
                (0, 36): [("x", 1, 4, 6)],
                (0, 38): [("x", 1, 6, 8), ("emaskd",), ("ident",)],
                (1, 0): [("x", 1, 8, 10)],
                (1, 2): [("x", 1, 10, 12)],
                (1, 4): [("x", 1, 12, 14), ("wq1", 0, 16)],
                (1, 6): [("x", 1, 14, 16)],
                (1, 8): [("x", 1, 16, 18), ("wq1", 16, 32)],
                (1, 10): [("x", 1, 18, 20)],
                (1, 12): [("x", 1, 20, 22), ("wq2", 0, 16)],
                (1, 14): [("x", 1, 22, 24)],
                (1, 16): [("x", 1, 24, 26), ("wq2", 16, 32)],
                (1, 18): [("x", 1, 26, 28)],
                (1, 20): [("x", 1, 28, 30), ("wq3", 0, 16)],
                (1, 22): [("x", 1, 30, 32)],
                (1, 24): [("wq3", 16, 32)],
            }

            for sh in range(2):
                k_ps, q_ps, v_ps = P[3 * sh], P[1 + 3 * sh], P[2 + 3 * sh]
                s0 = 512 * sh
                for d in range(DT + VLAG):
                    for ev in sched.get((sh, d), []):
                        issue_dma(ev)
                    if d < DT:
                        nc.tensor.matmul(k_ps[:], wk_sb[:, 128 * d:128 * (d + 1)],
                                         xs(sh, d), start=(d == 0),
                                         stop=(d == DT - 1))
                    dq = d - QLAG
                    if 0 <= dq < DT:
                        nc.tensor.matmul(q_ps[:], wq_sb[0][:, 128 * dq:128 * (dq + 1)],
                                         xs(sh, dq), start=(dq == 0),
                                         stop=(dq == DT - 1))
                    dv = d - VLAG
                    if dv >= 0:
                        nc.tensor.matmul(v_ps[:], wv_sb[:, 128 * dv:128 * (dv + 1)],
                                         xs(sh, dv), start=(dv == 0),
                                         stop=(dv == DT - 1))
                if sh == 0:
                    rope(k_ps, k_rot[:, s0:s0 + 512], s0)
                    rope(q_ps, q_rot[0][:, s0:s0 + 512], s0)
                    nc.vector.tensor_copy(v_et[:, s0:s0 + 512], v_ps[:])
                else:
                    nc.vector.tensor_copy(v_et[:, s0:s0 + 512], v_ps[:])
                    rope(q_ps, q_rot[0][:, s0:s0 + 512], s0)
                    rope(k_ps, k_rot[:, s0:s0 + 512], s0)

            if phases < 2:
                nc.sync.dma_start(outT[0:128, 0:512], v_et[:, 0:1024].bitcast(bf16)[:, 0:512])
                return

            # sh0-half V transposes into P6/P7 (untouched so far). Their
            # v_et[:, 0:512] source was written early in phase-A sh1.
            for t in range(4):
                tr = P[6 + (t % 2)][:, 0:128]
                nc.tensor.matmul(tr, v_et[:, 128 * t:128 * (t + 1)], ident_sb[:],
                                 is_transpose=True)
                nc.vector.tensor_copy(v_te[:, 128 * t:128 * (t + 1)], tr)

            if phases == 2:
                nc.sync.dma_start(outT[0:128, :], v_te[:])
                return

            # ---------------- pipeline slots ----------------
            UNITS = [(0, 0), (1, 0), (2, 0), (3, 0), (0, 1), (1, 1), (2, 1),
                     (3, 1), (4, 0), (4, 1), (5, 0), (5, 1), (6, 0), (6, 1),
                     (7, 0), (7, 1)]
            PV_DELAY = 3

            def expm_of(h, t):
                return expmb[(8 * h + t) % 4]

            def emit_sc_exp(h, i, t, c):
                sc = P[6 + (i % 2)]
                lo, hi = 512 * c, 512 * (c + 1)
                dlo, dhi = 128 * t, 128 * (t + 1)
                nc.tensor.matmul(sc[:], k_rot[:, dlo:dhi], q_rot[h][:, lo:hi],
                                 start=True, stop=True)
                expm = expm_of(h, t)
                if dlo >= hi:
                    nc.scalar.activation(expm[:, lo:hi], sc[:], Exp,
                                         scale=inv_sqrt_hd)
                elif dhi <= lo:
                    nc.scalar.activation(expm[:, lo:hi], sc[:], Exp,
                                         scale=inv_sqrt_hd, bias=1.0)
                else:
                    nc.scalar.activation(expm[:, lo:hi], sc[:], Exp,
                                         scale=inv_sqrt_hd)
                    nc.gpsimd.tensor_tensor(
                        expm[:, dlo:dhi], expm[:, dlo:dhi],
                        emaskd_sb[:, 128 * t:128 * (t + 1)], op=mult)
                    if dhi < hi:
                        nc.gpsimd.tensor_scalar_mul(
                            expm[:, dhi:hi], expm[:, dhi:hi], float(np.e))

            def emit_pv(h, t, c):
                expm = expm_of(h, t)
                lo, hi = 512 * c, 512 * (c + 1)
                nc.tensor.matmul(P[c][:], v_te[:, 128 * t:128 * (t + 1)],
                                 expm[:, lo:hi], start=(t == 0),
                                 stop=(t == TT - 1))

            def emit_z(h, t, c):
                # single-shot per-(t, sb) partials at column 8t+sb: interleaved
                # open accumulation groups in one bank corrupt on HW, so the
                # t-sum happens later on DVE (3-step tree in tail 1)
                expm = expm_of(h, t)
                for sb in range(4 * c, 4 * c + 4):
                    nc.tensor.matmul(P[2][:, 8 * t + sb:8 * t + sb + 1],
                                     expm[:, 128 * sb:128 * (sb + 1)],
                                     onec_sb[:], start=True, stop=True,
                                     skip_group_check=True)

            def emit_qproj(hq, sh, d):
                bank = P[5 - sh]
                nc.tensor.matmul(bank[:], wq_sb[hq][:, 128 * d:128 * (d + 1)],
                                 xs(sh, d), start=(d == 0), stop=(d == DT - 1))
                if d == DT - 1:
                    rope(bank, q_rot[hq][:, 512 * sh:512 * sh + 512], 512 * sh,
                         use_act=False)

            def emit_tail2a(h, rz_cat):
                # scatter rz columns to partition-0 rows with 8 small
                # SBUF->SBUF DMAs (engines cannot read partitions 1..7, and
                # a PE [128,1]-transpose chain proved wrong on hardware)
                rz_bf = zpool.tile([128, 8], bf16, name="rz_bf", tag="rzbf",
                                   bufs=2)
                nc.scalar.copy(rz_bf[:], rz_cat[:])
                # one gather DMA (partition-outer order: dst[8p+sb]=rz_bf[p,sb])
                # instead of 8 per-column DMAs, which cost ~700ns of queue each
                rts_all = zpool.tile([1, 1024], bf16, name="rts_all",
                                     tag="rtsall", bufs=2)
                nc.sync.dma_start(rts_all[:], rz_bf[:])
                return rts_all

            def emit_tail2b(h, o_sb, rts):
                # broadcast rz across partitions into {P3 (c0), P2 (c1)},
                # then normalize the (already SBUF-decoupled) o into a[h].
                for sb in range(8):
                    nc.tensor.matmul(P[3 - (sb // 4)][:, 128 * (sb % 4):
                                                      128 * (sb % 4) + 128],
                                     oner_sb[:], rts[0:1, sb:1024:8],
                                     start=True, stop=True)
                nc.vector.tensor_tensor(a[h][:, 0:512], o_sb[:, 0:512],
                                        P[3][:], op=mult)
                nc.vector.tensor_tensor(a[h][:, 512:1024], o_sb[:, 512:1024],
                                        P[2][:], op=mult)
                if phases == 3:
                    rz_sb = zpool.tile([128, S], bf16, name="rz_sb",
                                       tag="rzsb", bufs=4)
                    dbg_rz.append(rz_sb)
                    nc.scalar.copy(rz_sb[:, 0:512], P[3][:])
                    nc.vector.tensor_copy(rz_sb[:, 512:1024], P[2][:])

            NHEADS = NREP if phases >= 3 else 0
            dbg_rz = []
            pend_tail = None
            for h in range(NHEADS):
                qp = ([(h + 1, sh, d) for sh in range(2) for d in range(DT)]
                      if h + 1 < NREP else [])
                qi = 0
                if h < 2:
                    issue_dma(("wo", 2 * h, 0))
                    issue_dma(("wo", 2 * h + 1, 0))
                tail_a_i, tail_b_i = 2, 5
                tail_done = pend_tail is None
                pend_z = []
                fillers = ([(hh, c) for hh in range(3) for c in range(2)]
                           if h == NREP - 1 and phases >= 4 else [])
                units = UNITS
                for i, (t, c) in enumerate(units):
                    emit_sc_exp(h, i, t, c)
                    take = min(4, len(qp) - qi)
                    for _ in range(take):
                        emit_qproj(*qp[qi])
                        qi += 1
                    if h == 0 and 3 <= i <= 6:
                        # sh1-half V transposes (v_et source long ready)
                        t4 = i + 1  # 4..7
                        tr = P[3 + (i % 2)][:, 128:256]
                        nc.tensor.matmul(tr, v_et[:, 128 * t4:128 * (t4 + 1)],
                                         ident_sb[:], is_transpose=True)
                        nc.vector.tensor_copy(v_te[:, 128 * t4:128 * (t4 + 1)],
                                              tr)
                    if h == NREP - 1 and 8 <= i <= 13 and fillers:
                        # fill the qproj-less last slot: pre-accumulate
                        # phase-4 do=0 over heads 0..2 into {P4 (c0), P5 (c1)}
                        hh, cf = fillers.pop(0)
                        nc.tensor.matmul(P[4 + cf][:],
                                         wo_half[(hh, 0)][:, 0:128],
                                         a[hh][:, 512 * cf:512 * (cf + 1)],
                                         start=(hh == 0), stop=False,
                                         skip_group_check=True)
                    if i == tail_a_i and pend_tail is not None:
                        pend_rts = emit_tail2a(pend_tail[0], pend_tail[1])
                    if i == tail_b_i and pend_tail is not None:
                        emit_tail2b(pend_tail[0], pend_tail[2], pend_rts)
                        tail_done = True
                        pend_tail = None
                    if i >= PV_DELAY:
                        emit_pv(h, *units[i - PV_DELAY])
                        # z matmuls write P2, which tail2b(h-1)'s broadcast
                        # also writes: hold them until tail2b has been emitted
                        pend_z.append(units[i - PV_DELAY])
                        if tail_done:
                            while pend_z:
                                emit_z(h, *pend_z.pop(0))
                for j in range(PV_DELAY, 0, -1):
                    emit_pv(h, *units[len(units) - j])
                    pend_z.append(units[len(units) - j])
                while pend_z:
                    emit_z(h, *pend_z.pop(0))
                while qi < len(qp):
                    emit_qproj(*qp[qi])
                    qi += 1

                # tail part 1: tree-sum the z partials (cols 8t+sb), then
                # rz = 1/(z + 1024); free the o banks immediately by copying
                # o to SBUF
                zp = zpool.tile([128, 64], f32, name="zp", tag="zp", bufs=2)
                nc.vector.tensor_copy(zp[:], P[2][:, 0:64])
                nc.vector.tensor_tensor(zp[:, 0:32], zp[:, 0:32], zp[:, 32:64],
                                        op=add)
                nc.vector.tensor_tensor(zp[:, 0:16], zp[:, 0:16], zp[:, 16:32],
                                        op=add)
                z2 = zpool.tile([128, 8], f32, name="z2", tag="z2", bufs=2)
                rz_cat = zpool.tile([128, 8], f32, name="rz_cat", tag="rzc",
                                    bufs=2)
                nc.vector.tensor_tensor(z2[:], zp[:, 0:8], zp[:, 8:16], op=add)
                nc.vector.tensor_scalar_add(z2[:], z2[:], float(MAXSEQ - S))
                nc.vector.reciprocal(rz_cat[:], z2[:])
                o_sb = zpool.tile([128, S], bf16, name="o_sb", tag="osbuf",
                                  bufs=2)
                nc.scalar.copy(o_sb[:, 0:512], P[0][:])
                nc.vector.tensor_copy(o_sb[:, 512:1024], P[1][:])
                pend_tail = (h, rz_cat, o_sb)
                if h == NHEADS - 1:
                    # issue the rz scatter DMAs now, but hold the bc matmuls:
                    # blocked matmuls fill the PE's 4-deep wait queue, so they
                    # must sit behind runnable phase-4 work (emitted below)
                    pend_rts = emit_tail2a(h, rz_cat)
                    last_tail = (h, o_sb, pend_rts)
                    pend_tail = None
                    if phases == 3:
                        emit_tail2b(*last_tail)

            if phases == 3:
                for h in range(NREP):
                    nc.sync.dma_start(outT[128 * h:128 * (h + 1), :], a[h][:])
                    nc.sync.dma_start(outT[128 * (4 + h):128 * (5 + h), :],
                                      dbg_rz[h][:])
                nc.sync.dma_start(outT[1024:1152, :], v_te[:])
                nc.sync.dma_start(outT[1152:1280, :], k_rot[:])
                nc.sync.dma_start(outT[1280:1408, :], q_rot[0][:])
                return

            # ---------------- phase 4: output projection ----------------
            PAIRS = [[P[4], P[5]], [P[6], P[7]], [P[0], P[1]]]

            def wo_mms(do, hh, start, stop):
                op = PAIRS[do % 3]
                half, dl = do // 16, do % 16
                w = wo_half[(hh, half)][:, 128 * dl:128 * (dl + 1)]
                for c_ in range(2):
                    nc.tensor.matmul(op[c_][:], w,
                                     a[hh][:, 512 * c_:512 * (c_ + 1)],
                                     start=start, stop=stop,
                                     skip_group_check=True)

            def wo_out(do):
                op = PAIRS[do % 3]
                if do < DT - 1:
                    out_sb = opool.tile([128, S], bf16, name="out_sb",
                                        tag="osb", bufs=3)
                    nc.vector.tensor_copy(out_sb[:, 0:512], op[0][:])
                    nc.scalar.copy(out_sb[:, 512:1024], op[1][:])
                    nc.sync.dma_start(outT[128 * do:128 * (do + 1), :],
                                      out_sb[:])
                else:
                    # dedicated buffers so the final copies never wait on the
                    # out_sb/DMA rotation
                    oA = opool.tile([128, 512], bf16, name="oA", tag="oA",
                                    bufs=1)
                    oB = opool.tile([128, 512], bf16, name="oB", tag="oB",
                                    bufs=1)
                    nc.vector.tensor_copy(oA[:], op[0][:])
                    nc.sync.dma_start(outT[128 * do:128 * (do + 1), 0:512],
                                      oA[:])
                    nc.scalar.copy(oB[:], op[1][:])
                    nc.sync.dma_start(outT[128 * do:128 * (do + 1), 512:1024],
                                      oB[:])

            issue_dma(("wo", 0, 1))
            issue_dma(("wo", 1, 1))
            # a[3] arrives late (head 3's rz scatter DMAs trail the last
            # slot), so run do 1-2 over heads 0-2 first; do 0's heads 0-2
            # were pre-accumulated as last-slot fillers.
            for do in (1, 2):
                for hh in range(3):
                    wo_mms(do, hh, start=(hh == 0), stop=False)
            emit_tail2b(*last_tail)
            for do in (0, 1, 2):
                wo_mms(do, 3, start=False, stop=True)
                wo_out(do)
            for do in range(3, DT):
                if do == 4:
                    issue_dma(("wo", 2, 1))
                    issue_dma(("wo", 3, 1))
                for hh in range(NREP):
                    wo_mms(do, hh, start=(hh == 0), stop=(hh == NREP - 1))
                wo_out(do)

        for _rep in range(repeat):
            _body()

    nc.compile()
    return nc


def kernel(**inputs):
    import ml_dtypes
    from concourse.bass_utils import run_bass_kernel_spmd

    bf = ml_dtypes.bfloat16
    x = np.asarray(inputs["x"], np.float32)                 # [1, S, D]
    cos = np.asarray(inputs["freqs_cos"], np.float32)       # [S, 64]
    sin = np.asarray(inputs["freqs_sin"], np.float32)       # [S, 64]
    wq = np.asarray(inputs["wq"], np.float32)               # [NH, HD, D]
    wk = np.asarray(inputs["wk"], np.float32)               # [NKV, HD, D]
    wv = np.asarray(inputs["wv"], np.float32)               # [NKV, HD, D]
    wo = np.asarray(inputs["wo"], np.float32)               # [D, D]
    input_pos = np.asarray(inputs["input_pos"]).astype(np.int64)  # [S]

    if "nc" not in _CACHE:
        _CACHE["nc"] = _build_nc()
    nc = _CACHE["nc"]

    perm = np.concatenate([np.arange(0, HD, 2), np.arange(1, HD, 2)])
    # x: [D, S] -> sh-major pack [128, 2*16384]
    xT = np.ascontiguousarray(
        x[0].T.reshape(DT, 128, 2, 512).transpose(1, 2, 0, 3)
        .reshape(128, 2 * DT * 512)).astype(bf)
    cc = np.ascontiguousarray(np.concatenate([cos.T, cos.T], 0)).astype(bf)
    ns = np.ascontiguousarray(np.concatenate([-sin.T, sin.T], 0)).astype(bf)
    # visibility adds +1 pre-exp where input_pos[t] <= input_pos[s]; for the
    # (spec-guaranteed) sorted arange fill only diagonal blocks are mixed.
    emaskd_t = np.empty((TT, 128, 128), np.float32)
    for t in range(TT):
        p = input_pos[128 * t:128 * (t + 1)]
        emaskd_t[t] = np.where(p[:, None] <= p[None, :], np.float32(np.e),
                               np.float32(1.0))
    emaskd = np.ascontiguousarray(
        emaskd_t.transpose(1, 0, 2).reshape(128, TT * 128)).astype(bf)
    ident = np.eye(128, dtype=np.float32)

    def pmajor(wT):
        # [D, 128e] -> [128p, DT*128e] partition-major
        return np.ascontiguousarray(
            wT.reshape(DT, 128, HD).transpose(1, 0, 2).reshape(128, DT * HD))

    in_maps = []
    for g in range(NCORES):
        wq_g = wq[NREP * g:NREP * (g + 1)][:, perm, :]       # [4, 128, D]
        wo_g = wo[:, NREP * HD * g:NREP * HD * (g + 1)].T    # [512, D]
        in_maps.append({
            "xT": xT,
            "wq_t": np.stack([pmajor(wq_g[j].T) for j in range(NREP)]).astype(bf),
            "wk_t": pmajor(wk[g][perm].T).astype(bf),
            "wv_t": pmajor(wv[g].T).astype(bf),
            "wo_t": np.ascontiguousarray(
                wo_g.reshape(NREP, 128, D).transpose(1, 0, 2)
                .reshape(128, NREP * D)).astype(bf),
            "cc": cc, "ns": ns, "emaskd": emaskd, "ident": ident,
        })

    res = run_bass_kernel_spmd(nc, in_maps, list(range(NCORES)))
    total = np.zeros((D, S), np.float32)
    for g in range(NCORES):
        total += np.asarray(res.results[g]["outT"], dtype=np.float32)
    return np.ascontiguousarray(total.T)[None]   # [1, S, D]
